# revision 38
# baseline (speedup 1.0000x reference)
"""Trainium2 Bass kernel for nn_LCN (locally-connected network).

Computation:
  x: (512, 1, 280, 280) -> non-overlapping 28x28 patches (10x10 grid, P=100)
  y[b, f, p] = sum_{k,l} x[b, 28ph+k, 28pw+l] * w[f*100+p, 0, k, l]
  y = relu(y + bias[f*100+p]);  out = y_flat @ dec_w.T + dec_b   (j = f*100 + p)

Strategy (v3): the problem is HBM-stream-bound, so minimize bytes moved and
keep the device program trivial:
  - Host casts x to bf16 (error budget 2e-2 >> bf16's ~5e-3; PE computed in
    bf16 anyway) -> halves the dominant x stream vs fp32.
  - Host pre-transposes x into contraction-major chunk layout so the PE
    does NO transposes / im2col: just matmuls, straight off the DMA stream.
  - DMA details that matter (measured): bf16-typed DMA runs at half the
    AXI beat width, so all streams are f32-typed and bitcast to bf16 for
    the PE; and only 128-partition-row DMAs align the 16 SDMA engines 1:1
    with the 16 SBUF AXI ports (112 rows -> pairwise port contention,
    16 vs 25 GB/s/engine). Hence each patch's 784 pixels are split as
    6 full K=128 chunks + a 16-pixel tail; tails of 4 patches pack into
    the 4 32-row strips of one column block and are applied by diagonal
    row+col-tiled K=16 matmuls.
  - Sharding: 2 batch halves (256 images) x 4 patch quarters (25 patches).
    Per core: x 9.8+0.5 MB + w 0.7 MB -> ~32us at ~343 GB/s/NC measured.
  - Mains: per patch 6 accumulating bf16 matmuls lhsT=w[128,16],
    rhs=x[128,256] -> y PSUM, 4 patches per PSUM tile at col offsets
    0/32/64/96 (tile_position -> concurrent col-tile streams), + 1 tail
    matmul lhsT=wt[16,16] at tile_position (32q,32q).
  - ACT: relu(y + bias) -> y_sb bf16; decoder matmul per group deferred by
    one group so the relu never stalls the PE; accumulated in PSUM.
  - The 1-patch group is processed first so the last DMA slice finishes
    into a wide 4-way-concurrent matmul tail.
Host sums the 4 patch-quarter partial decoder outputs and adds dec_b.
"""

import sys

import numpy as np

for _p in ("/opt/trn_rl_repo", "/opt/trn_rl_repo/concourse"):
    if _p not in sys.path:
        sys.path.insert(0, _p)

import concourse.bass as bass
import concourse.mybir as mybir
import concourse.tile as tile
from concourse import bacc

F32 = mybir.dt.float32
BF16 = mybir.dt.bfloat16

# Problem constants
B, H, W = 512, 280, 280
KS = 28
HS = WS = 10
P = 100         # patches per image
F = 16
OUT = 10
NCORES = 8

# Sharding: 2 batch halves x 4 patch quarters
NB = 256        # images per core
NP = 25         # patches per core
TM = 6          # full K=128 chunks per patch (768 of 784 pixels)
TAIL = 16       # leftover pixels per patch (pixel 768..783)
NG = (NP + 3) // 4  # PSUM groups of 4 patches (last group: 1 patch)
NXB = (NP + 7) // 8  # x-tail column blocks (8 patch tails per 128-row block)
YB = 3          # y PSUM bufs


def build_program():
    nc = bacc.Bacc("TRN2")
    # All streams are f32-typed (bf16 data, bitcast on the PE side) and use
    # 128 partition rows -- both required for full DMA rate (see docstring).
    # Everything that isn't the main x stream (w, w-tail, bias, dec, x-tail)
    # is packed into ONE const tensor: separate small DMAs decay into
    # 128 descriptor-dominated packets that crawl behind the x stream and
    # stall the in-order PE queue at the first decoder matmul.
    # x tails are packed DENSE: patch pl's 16 tail pixels at rows
    # [32*((pl%8)//2) + 16*(pl%2)] of column block pl//8; the K=32 tail
    # matmul covers two patches' rows, with the other patch's 16 rows
    # zeroed in the w operand.
    CW = NP * TM * F // 2       # 1200  w main (f32 cols)
    CWT = NP * F // 2           # 200   w tail (K=32 halves, bf16 -> f32 cols)
    CB = NG                     # 7     bias (real f32)
    CD = NG * OUT // 2          # 35    dec (bf16 pairs)
    CXT = NXB * NB // 2         # 512   x tail
    CC = CW + CWT + CB + CD + CXT
    c_d = nc.dram_tensor("consts", [128, CC], F32, kind="ExternalInput")
    x_d = nc.dram_tensor("x", [128, NP * TM * NB // 2], F32, kind="ExternalInput")
    o_d = nc.dram_tensor("out", [OUT, NB], F32, kind="ExternalOutput")

    with tile.TileContext(nc) as tc:
        with (
            tc.tile_pool(name="const", bufs=1) as constp,
            tc.tile_pool(name="xs", bufs=1) as xsp,
            tc.tile_pool(name="yps", bufs=YB, space="PSUM") as ypsp,
            tc.tile_pool(name="ops", bufs=1, space="PSUM") as opsp,
            tc.tile_pool(name="warm", bufs=1, space="PSUM") as warmp,
        ):
            c_sb = constp.tile([128, CC], F32)
            y_sb = constp.tile([128, NG * NB], BF16)
            zero_sb = constp.tile([128, NB], F32)
            out_sb = constp.tile([OUT, NB], F32)
            out_ps = opsp.tile([OUT, NB], F32)

            # consts first on the sync ring: one efficient wide DMA
            nc.sync.dma_start(out=c_sb[:], in_=c_d[:])
            nc.gpsimd.memset(zero_sb[:], 0.0)

            # HAM warm-up: bf16 matmuls on varied (iota) data into scratch
            # PSUM while the PE would otherwise idle waiting for the first x
            # slice. Unthrottles the PE clock gate (1.2 -> 2.4 GHz). fp32 or
            # all-zero matmuls do NOT register as PE activity.
            iota_sb = constp.tile([128, NB], mybir.dt.int32)
            nc.gpsimd.iota(iota_sb[:], [[1, NB]], channel_multiplier=77)
            iota_bf = iota_sb[:].bitcast(BF16)
            warm_ps = warmp.tile([128, 2 * NB], F32)

            def warm(n):
                for _ in range(n):
                    nc.tensor.matmul(
                        warm_ps[0:F, :], iota_bf[:, 0:F], iota_bf[:],
                        start=True, stop=True)

            warm(10)

            # groups processed 1-patch group first, so the final DMA slice
            # drains into a wide concurrent matmul tail; the last group is
            # DMA'd in two 2-patch slices so its matmuls overlap the stream
            order = [NG - 1] + list(range(NG - 1))
            slices = []
            for g in order:
                npg = min(4, NP - 4 * g)
                if g == order[-1]:
                    slices.append((g, 0, 2))
                    slices.append((g, 2, npg))
                else:
                    slices.append((g, 0, npg))

            x_tiles = {}
            for g, qlo, qhi in slices:
                t = xsp.tile([128, (qhi - qlo) * TM * NB // 2], F32,
                             name=f"xg{g}_{qlo}")
                off = (4 * g + qlo) * TM * NB // 2
                nc.sync.dma_start(
                    out=t[:], in_=x_d[:, off:off + (qhi - qlo) * TM * NB // 2])
                x_tiles[(g, qlo)] = t
            w_bf = c_sb[:, 0:CW].bitcast(BF16)
            wt_bf = c_sb[:, CW:CW + CWT].bitcast(BF16)
            bias_sb = c_sb[:, CW + CWT:CW + CWT + CB]
            dec_bf = c_sb[:, CW + CWT + CB:CW + CWT + CB + CD].bitcast(BF16)
            xt_bf = c_sb[:, CW + CWT + CB + CD:CC].bitcast(BF16)

            def emit_dec(g, first, last):
                nc.tensor.matmul(
                    out_ps[:],
                    dec_bf[:, g * OUT:(g + 1) * OUT],
                    y_sb[:, g * NB:(g + 1) * NB],
                    start=first,
                    stop=last,
                )

            prev_g = None
            for idx, g in enumerate(order):
                npg = min(4, NP - 4 * g)
                yt = ypsp.tile([128, NB], F32, name="y_ps")
                if idx < YB:
                    # clear stale/NaN PSUM so the gap rows are finite zeros
                    nc.vector.tensor_copy(yt[:], zero_sb[:])
                for g2, qlo, qhi in [s for s in slices if s[0] == g]:
                    xg_bf = x_tiles[(g, qlo)][:].bitcast(BF16)
                    for t in range(TM):
                        for q in range(qlo, qhi):
                            pl = 4 * g + q
                            nc.tensor.matmul(
                                yt[32 * q:32 * q + F, :],
                                w_bf[:, (pl * TM + t) * F:
                                     (pl * TM + t + 1) * F],
                                xg_bf[:, ((q - qlo) * TM + t) * NB:
                                      ((q - qlo) * TM + t + 1) * NB],
                                start=(t == 0),
                                stop=False,
                                tile_position=(0, 32 * q),
                            )
                    for q in range(qlo, qhi):
                        # 16-pixel tail: K=32 matmul over the dense-packed
                        # tail strip (covers this patch + its pair patch,
                        # whose rows are zero in the w operand)
                        pl = 4 * g + q
                        a2, blk = (pl % 8) // 2, pl // 8
                        nc.tensor.matmul(
                            yt[32 * q:32 * q + F, :],
                            wt_bf[32 * a2:32 * a2 + 32, pl * F:(pl + 1) * F],
                            xt_bf[32 * a2:32 * a2 + 32, blk * NB:(blk + 1) * NB],
                            start=False,
                            stop=True,
                            tile_position=(32 * a2, 32 * q),
                        )
                    if g == order[-1] and qhi < npg:
                        # keep the PE clock warm across the inter-sub-slice
                        # DMA wait before the final tail
                        warm(8)
                if g != order[-1]:
                    nc.scalar.activation(
                        out=y_sb[:, g * NB:(g + 1) * NB],
                        in_=yt[:],
                        func=mybir.ActivationFunctionType.Relu,
                        bias=bias_sb[:, g:g + 1],
                    )
                    # decoder deferred one group so its y_sb dependency (the
                    # relu above) never stalls the PE behind independent mains
                    if prev_g is not None:
                        emit_dec(prev_g, prev_g == order[0], False)
                    prev_g = g
                    if idx >= 3:
                        # fillers across the slice-wait gaps of the later
                        # groups so the PE clock is still warm for the tail
                        warm(8)
                else:
                    # final group: flush the deferred decoder, then pipeline
                    # relu -> dec -> psum-copy in column halves to shorten
                    # the serial dependency chain at the kernel tail
                    if prev_g is not None:
                        emit_dec(prev_g, prev_g == order[0], False)
                    for h in range(2):
                        cols = slice(g * NB + h * 128, g * NB + h * 128 + 128)
                        nc.scalar.activation(
                            out=y_sb[:, cols],
                            in_=yt[:, h * 128:h * 128 + 128],
                            func=mybir.ActivationFunctionType.Relu,
                            bias=bias_sb[:, g:g + 1],
                        )
                        nc.tensor.matmul(
                            out_ps[:, h * 128:h * 128 + 128],
                            dec_bf[:, g * OUT:(g + 1) * OUT],
                            y_sb[:, cols],
                            start=False,
                            stop=True,
                        )
                        nc.vector.tensor_copy(
                            out_sb[:, h * 128:h * 128 + 128],
                            out_ps[:, h * 128:h * 128 + 128])

            nc.sync.dma_start(out=o_d[:], in_=out_sb[:])

    return nc


def make_in_maps(x, weight, bias, dec_w):
    import ml_dtypes
    bf16 = ml_dtypes.bfloat16

    x = np.asarray(x, np.float32).reshape(B, H, W)
    xb = x.astype(bf16)
    # (bh, b, hs, k, ws, l) -> (bh, hs, ws, k, l, b): per-patch pixel-major
    x6 = xb.reshape(2, NB, HS, KS, WS, KS)
    pp = np.ascontiguousarray(x6.transpose(0, 2, 4, 3, 5, 1)).reshape(
        2, P, KS * KS, NB)
    # main chunks: pixel j in [0,768) -> row j%128, col (p, t=j//128, b)
    xm = np.ascontiguousarray(
        pp[:, :, :TM * 128].reshape(2, P, TM, 128, NB).transpose(0, 3, 1, 2, 4))
    tl = pp[:, :, TM * 128:]                      # (2, P, 16, NB)

    # weight row j = f*100 + p (pixel-major k*28+l)
    w3 = np.asarray(weight, np.float32).reshape(F, P, KS * KS)
    wm = np.ascontiguousarray(
        w3[:, :, :TM * 128].reshape(F, P, TM, 128).transpose(3, 1, 2, 0)
    ).astype(bf16)                                # (128, P, TM, F)

    b2 = np.asarray(bias, np.float32).reshape(F, P)
    d3 = np.asarray(dec_w, np.float32).reshape(OUT, F, P)

    in_maps = []
    for core in range(NCORES):
        bh, q4 = core // 4, core % 4
        p0 = q4 * NP
        xs = np.ascontiguousarray(xm[bh, :, p0:p0 + NP]).reshape(
            128, NP * TM * NB)
        ws = np.ascontiguousarray(wm[:, p0:p0 + NP]).reshape(
            128, NP * TM * F)
        xtl = np.zeros((128, NXB * NB), bf16)
        wtl = np.zeros((128, NP * F), bf16)
        bst = np.zeros((128, NG), np.float32)
        dst = np.zeros((128, NG * OUT), np.float32)
        for pl in range(NP):
            G, q = pl // 4, pl % 4
            r0 = 32 * ((pl % 8) // 2) + TAIL * (pl % 2)
            xtl[r0:r0 + TAIL, (pl // 8) * NB:(pl // 8 + 1) * NB] = \
                tl[bh, p0 + pl]
            wtl[r0:r0 + TAIL, pl * F:(pl + 1) * F] = \
                w3[:, p0 + pl, TM * 128:].T.astype(bf16)
            bst[32 * q:32 * q + F, G] = b2[:, p0 + pl]
            dst[32 * q:32 * q + F, G * OUT:(G + 1) * OUT] = d3[:, :, p0 + pl].T
        consts = np.concatenate(
            [ws.view(np.float32), wtl.view(np.float32), bst,
             dst.astype(bf16).view(np.float32), xtl.view(np.float32)],
            axis=1)
        in_maps.append({"consts": consts, "x": xs.view(np.float32)})
    return in_maps


def combine(results, dec_b):
    dec_b = np.asarray(dec_b, np.float32)
    out = np.zeros((B, OUT), np.float32)
    for bh in range(2):
        part = results[4 * bh]["out"].astype(np.float32)
        for q4 in range(1, 4):
            part = part + results[4 * bh + q4]["out"]
        out[bh * NB:(bh + 1) * NB] = part.T + dec_b
    return out


_cache = {}


def _get_nc():
    if "nc" not in _cache:
        nc = build_program()
        nc.finalize()
        _cache["nc"] = nc
    return _cache["nc"]


def _install_ntff_hook():
    """Provide the missing antenv.axon_hooks module so trace=True works
    under axon (replicates trn_boot._ntff_profile_via_ctypes)."""
    import contextlib
    import ctypes
    import types

    if "antenv.axon_hooks" in sys.modules:
        return
    so_path = "/opt/axon/libaxon_pjrt.so"
    holder = {}
    mod = types.ModuleType("antenv.axon_hooks")
    mod.set_axon_ntff_profile_hook = lambda h: holder.__setitem__("h", h)
    mod.get_axon_ntff_profile_hook = lambda: holder.get("h")
    sys.modules["antenv.axon_hooks"] = mod
    try:
        import antenv
        antenv.axon_hooks = mod
    except ImportError:
        pass

    lib = ctypes.CDLL(so_path)
    if not hasattr(lib, "axon_start_nrt_profile"):
        return
    lib.axon_start_nrt_profile.argtypes = [
        ctypes.POINTER(ctypes.c_int64), ctypes.c_size_t]
    lib.axon_start_nrt_profile.restype = ctypes.c_int64
    lib.axon_stop_nrt_profile.argtypes = [ctypes.c_char_p]
    lib.axon_stop_nrt_profile.restype = ctypes.c_int64

    @contextlib.contextmanager
    def _hook(output_dir, device_ids):
        import jax
        jax.devices()
        if device_ids:
            ids = (ctypes.c_int64 * len(device_ids))(*device_ids)
            rc = lib.axon_start_nrt_profile(ids, len(device_ids))
        else:
            rc = lib.axon_start_nrt_profile(None, 0)
        if rc != 0:
            raise RuntimeError(f"axon_start_nrt_profile rc={rc}")
        try:
            yield
        finally:
            n = lib.axon_stop_nrt_profile(str(output_dir).encode())
            print(f"profile: {n} file(s) written to {output_dir}")

    mod.set_axon_ntff_profile_hook(_hook)


def run(x, weight, bias, dec_w, dec_b, trace=False):
    from concourse import bass_utils
    from concourse.bass_utils import run_bass_kernel_spmd

    if trace:
        _install_ntff_hook()
        # artifact upload needs a bucket that doesn't exist here
        bass_utils.upload_artifacts = lambda tmpdir: tmpdir

    nc = _get_nc()
    in_maps = make_in_maps(x, weight, bias, dec_w)
    r = run_bass_kernel_spmd(nc, in_maps, list(range(NCORES)), trace=trace)
    return combine(r.results, dec_b), r


def kernel(x, weight, bias, dec_w, dec_b):
    out, _ = run(x, weight, bias, dec_w, dec_b, trace=False)
    return out


# revision 39
# speedup vs baseline: 1.1431x; 1.1431x over previous
"""Trainium2 Bass kernel for nn_LCN (locally-connected network).

Computation:
  x: (512, 1, 280, 280) -> non-overlapping 28x28 patches (10x10 grid, P=100)
  y[b, f, p] = sum_{k,l} x[b, 28ph+k, 28pw+l] * w[f*100+p, 0, k, l]
  y = relu(y + bias[f*100+p]);  out = y_flat @ dec_w.T + dec_b   (j = f*100 + p)

Strategy (v3): the problem is HBM-stream-bound, so minimize bytes moved and
keep the device program trivial:
  - Host casts x to bf16 (error budget 2e-2 >> bf16's ~5e-3; PE computed in
    bf16 anyway) -> halves the dominant x stream vs fp32.
  - Host pre-transposes x into contraction-major chunk layout so the PE
    does NO transposes / im2col: just matmuls, straight off the DMA stream.
  - DMA details that matter (measured): bf16-typed DMA runs at half the
    AXI beat width, so all streams are f32-typed and bitcast to bf16 for
    the PE; and only 128-partition-row DMAs align the 16 SDMA engines 1:1
    with the 16 SBUF AXI ports (112 rows -> pairwise port contention,
    16 vs 25 GB/s/engine). Hence each patch's 784 pixels are split as
    6 full K=128 chunks + a 16-pixel tail; tails of 4 patches pack into
    the 4 32-row strips of one column block and are applied by diagonal
    row+col-tiled K=16 matmuls.
  - Sharding: 2 batch halves (256 images) x 4 patch quarters (25 patches).
    Per core: x 9.8+0.5 MB + w 0.7 MB -> ~32us at ~343 GB/s/NC measured.
  - Mains: per patch 6 accumulating bf16 matmuls lhsT=w[128,16],
    rhs=x[128,256] -> y PSUM, 4 patches per PSUM tile at col offsets
    0/32/64/96 (tile_position -> concurrent col-tile streams), + 1 tail
    matmul lhsT=wt[16,16] at tile_position (32q,32q).
  - ACT: relu(y + bias) -> y_sb bf16; decoder matmul per group deferred by
    one group so the relu never stalls the PE; accumulated in PSUM.
  - The 1-patch group is processed first so the last DMA slice finishes
    into a wide 4-way-concurrent matmul tail.
Host sums the 4 patch-quarter partial decoder outputs and adds dec_b.
"""

import sys

import numpy as np

for _p in ("/opt/trn_rl_repo", "/opt/trn_rl_repo/concourse"):
    if _p not in sys.path:
        sys.path.insert(0, _p)

import concourse.bass as bass
import concourse.mybir as mybir
import concourse.tile as tile
from concourse import bacc

F32 = mybir.dt.float32
BF16 = mybir.dt.bfloat16

# Problem constants
B, H, W = 512, 280, 280
KS = 28
HS = WS = 10
P = 100         # patches per image
F = 16
OUT = 10
NCORES = 8

# Sharding: 2 batch halves x 4 patch quarters
NB = 256        # images per core
NP = 25         # patches per core
TM = 6          # full K=128 chunks per patch (768 of 784 pixels)
TAIL = 16       # leftover pixels per patch (pixel 768..783)
NG = (NP + 3) // 4  # PSUM groups of 4 patches (last group: 1 patch)
NXB = (NP + 7) // 8  # x-tail column blocks (8 patch tails per 128-row block)
YB = 3          # y PSUM bufs


def build_program():
    nc = bacc.Bacc("TRN2")
    # All streams are f32-typed (bf16 data, bitcast on the PE side) and use
    # 128 partition rows -- both required for full DMA rate (see docstring).
    # Everything that isn't the main x stream (w, w-tail, bias, dec, x-tail)
    # is packed into ONE const tensor: separate small DMAs decay into
    # 128 descriptor-dominated packets that crawl behind the x stream and
    # stall the in-order PE queue at the first decoder matmul.
    # x tails are packed DENSE: patch pl's 16 tail pixels at rows
    # [32*((pl%8)//2) + 16*(pl%2)] of column block pl//8; the K=32 tail
    # matmul covers two patches' rows, with the other patch's 16 rows
    # zeroed in the w operand.
    CW = NP * TM * F // 2       # 1200  w main (f32 cols)
    CWT = NP * F // 2           # 200   w tail (K=32 halves, bf16 -> f32 cols)
    CB = NG                     # 7     bias (real f32)
    CD = NG * OUT // 2          # 35    dec (bf16 pairs)
    CXT = NXB * NB // 2         # 512   x tail
    CC = CW + CWT + CB + CD + CXT
    c_d = nc.dram_tensor("consts", [128, CC], F32, kind="ExternalInput")
    x_d = nc.dram_tensor("x", [128, NP * TM * NB // 2], F32, kind="ExternalInput")
    o_d = nc.dram_tensor("out", [OUT, NB], F32, kind="ExternalOutput")

    with tile.TileContext(nc) as tc:
        with (
            tc.tile_pool(name="const", bufs=1) as constp,
            tc.tile_pool(name="xs", bufs=1) as xsp,
            tc.tile_pool(name="yps", bufs=YB, space="PSUM") as ypsp,
            tc.tile_pool(name="ops", bufs=1, space="PSUM") as opsp,
            tc.tile_pool(name="warm", bufs=1, space="PSUM") as warmp,
        ):
            c_sb = constp.tile([128, CC], F32)
            y_sb = constp.tile([128, NG * NB], BF16)
            zero_sb = constp.tile([128, NB], F32)
            out_sb = constp.tile([OUT, NB], F32)
            out_ps = opsp.tile([OUT, NB], F32)

            # consts first on the sync ring: one efficient wide DMA
            nc.sync.dma_start(out=c_sb[:], in_=c_d[:])
            nc.gpsimd.memset(zero_sb[:], 0.0)

            # HAM warm-up: bf16 matmuls on varied (iota) data into scratch
            # PSUM while the PE would otherwise idle waiting for the first x
            # slice. Unthrottles the PE clock gate (1.2 -> 2.4 GHz). fp32 or
            # all-zero matmuls do NOT register as PE activity.
            iota_sb = constp.tile([128, NB], mybir.dt.int32)
            nc.gpsimd.iota(iota_sb[:], [[1, NB]], channel_multiplier=77)
            iota_bf = iota_sb[:].bitcast(BF16)
            warm_ps = warmp.tile([128, 2 * NB], F32)

            def warm(n):
                for _ in range(n):
                    nc.tensor.matmul(
                        warm_ps[0:F, :], iota_bf[:, 0:F], iota_bf[:],
                        start=True, stop=True)

            warm(10)

            # groups processed 1-patch group first, so the final DMA slice
            # drains into a wide concurrent matmul tail; the last group is
            # DMA'd in two 2-patch slices so its matmuls overlap the stream
            order = [NG - 1] + list(range(NG - 1))
            slices = []
            for g in order:
                npg = min(4, NP - 4 * g)
                if g == order[-1]:
                    slices.append((g, 0, 2))
                    slices.append((g, 2, npg))
                else:
                    slices.append((g, 0, npg))

            x_tiles = {}
            for g, qlo, qhi in slices:
                t = xsp.tile([128, (qhi - qlo) * TM * NB // 2], F32,
                             name=f"xg{g}_{qlo}")
                off = (4 * g + qlo) * TM * NB // 2
                nc.sync.dma_start(
                    out=t[:], in_=x_d[:, off:off + (qhi - qlo) * TM * NB // 2])
                x_tiles[(g, qlo)] = t
            w_bf = c_sb[:, 0:CW].bitcast(BF16)
            wt_bf = c_sb[:, CW:CW + CWT].bitcast(BF16)
            bias_sb = c_sb[:, CW + CWT:CW + CWT + CB]
            dec_bf = c_sb[:, CW + CWT + CB:CW + CWT + CB + CD].bitcast(BF16)
            xt_bf = c_sb[:, CW + CWT + CB + CD:CC].bitcast(BF16)

            def emit_dec(g, first, last):
                nc.tensor.matmul(
                    out_ps[:],
                    dec_bf[:, g * OUT:(g + 1) * OUT],
                    y_sb[:, g * NB:(g + 1) * NB],
                    start=first,
                    stop=last,
                )

            prev_g = None
            for idx, g in enumerate(order):
                npg = min(4, NP - 4 * g)
                yt = ypsp.tile([128, NB], F32, name="y_ps")
                if idx < YB:
                    # clear stale/NaN PSUM so the gap rows are finite zeros
                    nc.vector.tensor_copy(yt[:], zero_sb[:])
                for g2, qlo, qhi in [s for s in slices if s[0] == g]:
                    xg_bf = x_tiles[(g, qlo)][:].bitcast(BF16)
                    for t in range(TM):
                        for q in range(qlo, qhi):
                            pl = 4 * g + q
                            nc.tensor.matmul(
                                yt[32 * q:32 * q + F, :],
                                w_bf[:, (pl * TM + t) * F:
                                     (pl * TM + t + 1) * F],
                                xg_bf[:, ((q - qlo) * TM + t) * NB:
                                      ((q - qlo) * TM + t + 1) * NB],
                                start=(t == 0),
                                stop=False,
                                tile_position=(0, 32 * q),
                            )
                    for q in range(qlo, qhi):
                        # 16-pixel tail: K=32 matmul over the dense-packed
                        # tail strip (covers this patch + its pair patch,
                        # whose rows are zero in the w operand)
                        pl = 4 * g + q
                        a2, blk = (pl % 8) // 2, pl // 8
                        nc.tensor.matmul(
                            yt[32 * q:32 * q + F, :],
                            wt_bf[32 * a2:32 * a2 + 32, pl * F:(pl + 1) * F],
                            xt_bf[32 * a2:32 * a2 + 32, blk * NB:(blk + 1) * NB],
                            start=False,
                            stop=True,
                            tile_position=(32 * a2, 32 * q),
                        )
                    if g == order[-1] and qhi < npg:
                        # keep the PE clock warm across the inter-sub-slice
                        # DMA wait before the final tail
                        warm(8)
                if g != order[-1]:
                    nc.scalar.activation(
                        out=y_sb[:, g * NB:(g + 1) * NB],
                        in_=yt[:],
                        func=mybir.ActivationFunctionType.Relu,
                        bias=bias_sb[:, g:g + 1],
                    )
                    # decoder deferred one group so its y_sb dependency (the
                    # relu above) never stalls the PE behind independent mains
                    if prev_g is not None:
                        emit_dec(prev_g, prev_g == order[0], False)
                    prev_g = g
                    if idx >= 1:
                        # fillers across the slice-wait gaps so the PE clock
                        # stays warm through the kernel tail
                        warm(8)
                else:
                    # final group: flush the deferred decoder, then pipeline
                    # relu -> dec -> psum-copy in column halves to shorten
                    # the serial dependency chain at the kernel tail
                    if prev_g is not None:
                        emit_dec(prev_g, prev_g == order[0], False)
                    for h in range(2):
                        cols = slice(g * NB + h * 128, g * NB + h * 128 + 128)
                        nc.scalar.activation(
                            out=y_sb[:, cols],
                            in_=yt[:, h * 128:h * 128 + 128],
                            func=mybir.ActivationFunctionType.Relu,
                            bias=bias_sb[:, g:g + 1],
                        )
                        nc.tensor.matmul(
                            out_ps[:, h * 128:h * 128 + 128],
                            dec_bf[:, g * OUT:(g + 1) * OUT],
                            y_sb[:, cols],
                            start=False,
                            stop=True,
                        )
                        nc.vector.tensor_copy(
                            out_sb[:, h * 128:h * 128 + 128],
                            out_ps[:, h * 128:h * 128 + 128])

            nc.sync.dma_start(out=o_d[:], in_=out_sb[:])

    return nc


def make_in_maps(x, weight, bias, dec_w):
    import ml_dtypes
    bf16 = ml_dtypes.bfloat16

    x = np.asarray(x, np.float32).reshape(B, H, W)
    xb = x.astype(bf16)
    # (bh, b, hs, k, ws, l) -> (bh, hs, ws, k, l, b): per-patch pixel-major
    x6 = xb.reshape(2, NB, HS, KS, WS, KS)
    pp = np.ascontiguousarray(x6.transpose(0, 2, 4, 3, 5, 1)).reshape(
        2, P, KS * KS, NB)
    # main chunks: pixel j in [0,768) -> row j%128, col (p, t=j//128, b)
    xm = np.ascontiguousarray(
        pp[:, :, :TM * 128].reshape(2, P, TM, 128, NB).transpose(0, 3, 1, 2, 4))
    tl = pp[:, :, TM * 128:]                      # (2, P, 16, NB)

    # weight row j = f*100 + p (pixel-major k*28+l)
    w3 = np.asarray(weight, np.float32).reshape(F, P, KS * KS)
    wm = np.ascontiguousarray(
        w3[:, :, :TM * 128].reshape(F, P, TM, 128).transpose(3, 1, 2, 0)
    ).astype(bf16)                                # (128, P, TM, F)

    b2 = np.asarray(bias, np.float32).reshape(F, P)
    d3 = np.asarray(dec_w, np.float32).reshape(OUT, F, P)

    in_maps = []
    for core in range(NCORES):
        bh, q4 = core // 4, core % 4
        p0 = q4 * NP
        xs = np.ascontiguousarray(xm[bh, :, p0:p0 + NP]).reshape(
            128, NP * TM * NB)
        ws = np.ascontiguousarray(wm[:, p0:p0 + NP]).reshape(
            128, NP * TM * F)
        xtl = np.zeros((128, NXB * NB), bf16)
        wtl = np.zeros((128, NP * F), bf16)
        bst = np.zeros((128, NG), np.float32)
        dst = np.zeros((128, NG * OUT), np.float32)
        for pl in range(NP):
            G, q = pl // 4, pl % 4
            r0 = 32 * ((pl % 8) // 2) + TAIL * (pl % 2)
            xtl[r0:r0 + TAIL, (pl // 8) * NB:(pl // 8 + 1) * NB] = \
                tl[bh, p0 + pl]
            wtl[r0:r0 + TAIL, pl * F:(pl + 1) * F] = \
                w3[:, p0 + pl, TM * 128:].T.astype(bf16)
            bst[32 * q:32 * q + F, G] = b2[:, p0 + pl]
            dst[32 * q:32 * q + F, G * OUT:(G + 1) * OUT] = d3[:, :, p0 + pl].T
        consts = np.concatenate(
            [ws.view(np.float32), wtl.view(np.float32), bst,
             dst.astype(bf16).view(np.float32), xtl.view(np.float32)],
            axis=1)
        in_maps.append({"consts": consts, "x": xs.view(np.float32)})
    return in_maps


def combine(results, dec_b):
    dec_b = np.asarray(dec_b, np.float32)
    out = np.zeros((B, OUT), np.float32)
    for bh in range(2):
        part = results[4 * bh]["out"].astype(np.float32)
        for q4 in range(1, 4):
            part = part + results[4 * bh + q4]["out"]
        out[bh * NB:(bh + 1) * NB] = part.T + dec_b
    return out


_cache = {}


def _get_nc():
    if "nc" not in _cache:
        nc = build_program()
        nc.finalize()
        _cache["nc"] = nc
    return _cache["nc"]


def _install_ntff_hook():
    """Provide the missing antenv.axon_hooks module so trace=True works
    under axon (replicates trn_boot._ntff_profile_via_ctypes)."""
    import contextlib
    import ctypes
    import types

    if "antenv.axon_hooks" in sys.modules:
        return
    so_path = "/opt/axon/libaxon_pjrt.so"
    holder = {}
    mod = types.ModuleType("antenv.axon_hooks")
    mod.set_axon_ntff_profile_hook = lambda h: holder.__setitem__("h", h)
    mod.get_axon_ntff_profile_hook = lambda: holder.get("h")
    sys.modules["antenv.axon_hooks"] = mod
    try:
        import antenv
        antenv.axon_hooks = mod
    except ImportError:
        pass

    lib = ctypes.CDLL(so_path)
    if not hasattr(lib, "axon_start_nrt_profile"):
        return
    lib.axon_start_nrt_profile.argtypes = [
        ctypes.POINTER(ctypes.c_int64), ctypes.c_size_t]
    lib.axon_start_nrt_profile.restype = ctypes.c_int64
    lib.axon_stop_nrt_profile.argtypes = [ctypes.c_char_p]
    lib.axon_stop_nrt_profile.restype = ctypes.c_int64

    @contextlib.contextmanager
    def _hook(output_dir, device_ids):
        import jax
        jax.devices()
        if device_ids:
            ids = (ctypes.c_int64 * len(device_ids))(*device_ids)
            rc = lib.axon_start_nrt_profile(ids, len(device_ids))
        else:
            rc = lib.axon_start_nrt_profile(None, 0)
        if rc != 0:
            raise RuntimeError(f"axon_start_nrt_profile rc={rc}")
        try:
            yield
        finally:
            n = lib.axon_stop_nrt_profile(str(output_dir).encode())
            print(f"profile: {n} file(s) written to {output_dir}")

    mod.set_axon_ntff_profile_hook(_hook)


def run(x, weight, bias, dec_w, dec_b, trace=False):
    from concourse import bass_utils
    from concourse.bass_utils import run_bass_kernel_spmd

    if trace:
        _install_ntff_hook()
        # artifact upload needs a bucket that doesn't exist here
        bass_utils.upload_artifacts = lambda tmpdir: tmpdir

    nc = _get_nc()
    in_maps = make_in_maps(x, weight, bias, dec_w)
    r = run_bass_kernel_spmd(nc, in_maps, list(range(NCORES)), trace=trace)
    return combine(r.results, dec_b), r


def kernel(x, weight, bias, dec_w, dec_b):
    out, _ = run(x, weight, bias, dec_w, dec_b, trace=False)
    return out


# revision 49
# speedup vs baseline: 1.1911x; 1.0420x over previous
"""Trainium2 Bass kernel for nn_LCN (locally-connected network).

Computation:
  x: (512, 1, 280, 280) -> non-overlapping 28x28 patches (10x10 grid, P=100)
  y[b, f, p] = sum_{k,l} x[b, 28ph+k, 28pw+l] * w[f*100+p, 0, k, l]
  y = relu(y + bias[f*100+p]);  out = y_flat @ dec_w.T + dec_b   (j = f*100 + p)

Strategy (v3): the problem is HBM-stream-bound, so minimize bytes moved and
keep the device program trivial:
  - Host casts x to bf16 (error budget 2e-2 >> bf16's ~5e-3; PE computed in
    bf16 anyway) -> halves the dominant x stream vs fp32.
  - Host pre-transposes x into contraction-major chunk layout so the PE
    does NO transposes / im2col: just matmuls, straight off the DMA stream.
  - DMA details that matter (measured): bf16-typed DMA runs at half the
    AXI beat width, so all streams are f32-typed and bitcast to bf16 for
    the PE; and only 128-partition-row DMAs align the 16 SDMA engines 1:1
    with the 16 SBUF AXI ports (112 rows -> pairwise port contention,
    16 vs 25 GB/s/engine). Hence each patch's 784 pixels are split as
    6 full K=128 chunks + a 16-pixel tail; tails of 4 patches pack into
    the 4 32-row strips of one column block and are applied by diagonal
    row+col-tiled K=16 matmuls.
  - Sharding: 2 batch halves (256 images) x 4 patch quarters (25 patches).
    Per core: x 9.8+0.5 MB + w 0.7 MB -> ~32us at ~343 GB/s/NC measured.
  - Mains: per patch 6 accumulating bf16 matmuls lhsT=w[128,16],
    rhs=x[128,256] -> y PSUM, 4 patches per PSUM tile at col offsets
    0/32/64/96 (tile_position -> concurrent col-tile streams), + 1 tail
    matmul lhsT=wt[16,16] at tile_position (32q,32q).
  - ACT: relu(y + bias) -> y_sb bf16; decoder matmul per group deferred by
    one group so the relu never stalls the PE; accumulated in PSUM.
  - The 1-patch group is processed first so the last DMA slice finishes
    into a wide 4-way-concurrent matmul tail.
Host sums the 4 patch-quarter partial decoder outputs and adds dec_b.
"""

import sys

import numpy as np

for _p in ("/opt/trn_rl_repo", "/opt/trn_rl_repo/concourse"):
    if _p not in sys.path:
        sys.path.insert(0, _p)

import concourse.bass as bass
import concourse.mybir as mybir
import concourse.tile as tile
from concourse import bacc

F32 = mybir.dt.float32
BF16 = mybir.dt.bfloat16
FP8 = mybir.dt.float8e4

# Problem constants
B, H, W = 512, 280, 280
KS = 28
HS = WS = 10
P = 100         # patches per image
F = 16
OUT = 10
NCORES = 8

# Sharding: 2 batch halves x 4 patch quarters
NB = 256        # images per core
NP = 25         # patches per core
TM = 6          # full K=128 chunks per patch (768 of 784 pixels)
TAIL = 16       # leftover pixels per patch (pixel 768..783)
NG = (NP + 3) // 4  # PSUM groups of 4 patches (last group: 1 patch)
NXB = (NP + 7) // 8  # x-tail column blocks (8 patch tails per 128-row block)
YB = 3          # y PSUM bufs

# Mixed precision: the first NT8 K=128 chunks of each patch are fp8-e4m3
# (256 of 784 pixels). Measured end-to-end rel err 0.0158 vs the 2e-2 gate
# (bf16-only: 0.0046); saves 1.64 MB/core of the HBM stream. w stays bf16.
NT8 = 2
NTM = TM - NT8
SLW = NT8 * NB // 4 + NTM * NB // 2  # 640 f32 cols per patch in x_d

# processing order (1-patch group first) and DMA slices (last group split)
ORDER = [NG - 1] + list(range(NG - 1))
SLICES = []
for _g in ORDER:
    _npg = min(4, NP - 4 * _g)
    if _g == ORDER[-1]:
        SLICES += [(_g, 0, 2), (_g, 2, _npg)]
    else:
        SLICES += [(_g, 0, _npg)]


def build_program():
    nc = bacc.Bacc("TRN2")
    # All streams are f32-typed (bf16 data, bitcast on the PE side) and use
    # 128 partition rows -- both required for full DMA rate (see docstring).
    # Everything that isn't the main x stream (w, w-tail, bias, dec, x-tail)
    # is packed into ONE const tensor: separate small DMAs decay into
    # 128 descriptor-dominated packets that crawl behind the x stream and
    # stall the in-order PE queue at the first decoder matmul.
    # x tails are packed DENSE: patch pl's 16 tail pixels at rows
    # [32*((pl%8)//2) + 16*(pl%2)] of column block pl//8; the K=32 tail
    # matmul covers two patches' rows, with the other patch's 16 rows
    # zeroed in the w operand.
    CW = NP * TM * F // 2       # 1200  w main (f32 cols)
    CWT = NP * F // 2           # 200   w tail (K=32 halves, bf16 -> f32 cols)
    CB = NG                     # 7     bias (real f32)
    CD = NG * OUT // 2          # 35    dec (bf16 pairs)
    CXT = NXB * NB // 2         # 512   x tail
    CC = CW + CWT + CB + CD + CXT
    c_d = nc.dram_tensor("consts", [128, CC], F32, kind="ExternalInput")
    x_d = nc.dram_tensor("x", [128, NP * SLW], F32, kind="ExternalInput")
    o_d = nc.dram_tensor("out", [OUT, NB], F32, kind="ExternalOutput")

    with tile.TileContext(nc) as tc:
        with (
            tc.tile_pool(name="const", bufs=1) as constp,
            tc.tile_pool(name="xs", bufs=1) as xsp,
            tc.tile_pool(name="yps", bufs=YB, space="PSUM") as ypsp,
            tc.tile_pool(name="ops", bufs=1, space="PSUM") as opsp,
            tc.tile_pool(name="warm", bufs=1, space="PSUM") as warmp,
        ):
            c_sb = constp.tile([128, CC], F32)
            y_sb = constp.tile([128, NG * NB], BF16)
            zero_sb = constp.tile([128, NB], F32)
            out_sb = constp.tile([OUT, NB], F32)
            out_ps = opsp.tile([OUT, NB], F32)

            # consts first on the sync ring: one efficient wide DMA
            nc.sync.dma_start(out=c_sb[:], in_=c_d[:])
            nc.gpsimd.memset(zero_sb[:], 0.0)

            # HAM warm-up: bf16 matmuls on varied (iota) data into scratch
            # PSUM while the PE would otherwise idle waiting for the first x
            # slice. Unthrottles the PE clock gate (1.2 -> 2.4 GHz). fp32 or
            # all-zero matmuls do NOT register as PE activity.
            iota_sb = constp.tile([128, NB], mybir.dt.int32)
            nc.gpsimd.iota(iota_sb[:], [[1, NB]], channel_multiplier=77)
            iota_bf = iota_sb[:].bitcast(BF16)
            warm_ps = warmp.tile([128, 2 * NB], F32)

            def warm(n):
                for _ in range(n):
                    nc.tensor.matmul(
                        warm_ps[0:F, :], iota_bf[:, 0:F], iota_bf[:],
                        start=True, stop=True)

            warm(10)

            # x slices in consumption order; each slice = [fp8 block | bf16
            # block] for its patches, one DMA per slice
            x_tiles = {}
            off = 0
            for g, qlo, qhi in SLICES:
                n = qhi - qlo
                t = xsp.tile([128, n * SLW], F32, name=f"xg{g}_{qlo}")
                nc.sync.dma_start(out=t[:], in_=x_d[:, off:off + n * SLW])
                x_tiles[(g, qlo)] = t
                off += n * SLW
            w_bf = c_sb[:, 0:CW].bitcast(BF16)
            wt_bf = c_sb[:, CW:CW + CWT].bitcast(BF16)
            bias_sb = c_sb[:, CW + CWT:CW + CWT + CB]
            dec_bf = c_sb[:, CW + CWT + CB:CW + CWT + CB + CD].bitcast(BF16)
            xt_bf = c_sb[:, CW + CWT + CB + CD:CC].bitcast(BF16)

            def emit_dec(g, first, last):
                nc.tensor.matmul(
                    out_ps[:],
                    dec_bf[:, g * OUT:(g + 1) * OUT],
                    y_sb[:, g * NB:(g + 1) * NB],
                    start=first,
                    stop=last,
                )

            prev_g = None
            for idx, g in enumerate(ORDER):
                npg = min(4, NP - 4 * g)
                yt = ypsp.tile([128, NB], F32, name="y_ps")
                if idx < YB:
                    # clear stale/NaN PSUM so the gap rows are finite zeros
                    nc.vector.tensor_copy(yt[:], zero_sb[:])
                for g2, qlo, qhi in [s for s in SLICES if s[0] == g]:
                    n = qhi - qlo
                    xg = x_tiles[(g, qlo)]
                    x8 = xg[:, 0:n * NT8 * NB // 4].bitcast(FP8)
                    xm = xg[:, n * NT8 * NB // 4:n * SLW].bitcast(BF16)
                    for t in range(TM):
                        for q in range(qlo, qhi):
                            pl = 4 * g + q
                            if t < NT8:
                                rhs = x8[:, ((q - qlo) * NT8 + t) * NB:
                                         ((q - qlo) * NT8 + t + 1) * NB]
                            else:
                                t2 = t - NT8
                                rhs = xm[:, ((q - qlo) * NTM + t2) * NB:
                                         ((q - qlo) * NTM + t2 + 1) * NB]
                            nc.tensor.matmul(
                                yt[32 * q:32 * q + F, :],
                                w_bf[:, (pl * TM + t) * F:
                                     (pl * TM + t + 1) * F],
                                rhs,
                                start=(t == 0),
                                stop=False,
                                tile_position=(0, 32 * q),
                            )
                    for q in range(qlo, qhi):
                        # 16-pixel tail: K=32 matmul over the dense-packed
                        # tail strip (covers this patch + its pair patch,
                        # whose rows are zero in the w operand)
                        pl = 4 * g + q
                        a2, blk = (pl % 8) // 2, pl // 8
                        nc.tensor.matmul(
                            yt[32 * q:32 * q + F, :],
                            wt_bf[32 * a2:32 * a2 + 32, pl * F:(pl + 1) * F],
                            xt_bf[32 * a2:32 * a2 + 32, blk * NB:(blk + 1) * NB],
                            start=False,
                            stop=True,
                            tile_position=(32 * a2, 32 * q),
                        )
                    if g == ORDER[-1] and qhi < npg:
                        # keep the PE clock warm across the inter-sub-slice
                        # DMA wait before the final tail
                        warm(8)
                if g != ORDER[-1]:
                    nc.scalar.activation(
                        out=y_sb[:, g * NB:(g + 1) * NB],
                        in_=yt[:],
                        func=mybir.ActivationFunctionType.Relu,
                        bias=bias_sb[:, g:g + 1],
                    )
                    # decoder deferred one group so its y_sb dependency (the
                    # relu above) never stalls the PE behind independent mains
                    if prev_g is not None:
                        emit_dec(prev_g, prev_g == ORDER[0], False)
                    prev_g = g
                    if idx >= 1:
                        # fillers across the slice-wait gaps so the PE clock
                        # stays warm through the kernel tail
                        warm(8)
                else:
                    # final group: flush the deferred decoder, then pipeline
                    # relu -> dec -> psum-copy in column halves to shorten
                    # the serial dependency chain at the kernel tail
                    if prev_g is not None:
                        emit_dec(prev_g, prev_g == ORDER[0], False)
                    for h in range(2):
                        cols = slice(g * NB + h * 128, g * NB + h * 128 + 128)
                        nc.scalar.activation(
                            out=y_sb[:, cols],
                            in_=yt[:, h * 128:h * 128 + 128],
                            func=mybir.ActivationFunctionType.Relu,
                            bias=bias_sb[:, g:g + 1],
                        )
                        nc.tensor.matmul(
                            out_ps[:, h * 128:h * 128 + 128],
                            dec_bf[:, g * OUT:(g + 1) * OUT],
                            y_sb[:, cols],
                            start=False,
                            stop=True,
                        )
                        nc.vector.tensor_copy(
                            out_sb[:, h * 128:h * 128 + 128],
                            out_ps[:, h * 128:h * 128 + 128])

            nc.sync.dma_start(out=o_d[:], in_=out_sb[:])

    return nc


def make_in_maps(x, weight, bias, dec_w):
    import ml_dtypes
    bf16 = ml_dtypes.bfloat16

    x = np.asarray(x, np.float32).reshape(B, H, W)
    xb = x.astype(bf16)
    # (bh, b, hs, k, ws, l) -> (bh, hs, ws, k, l, b): per-patch pixel-major
    x6 = xb.reshape(2, NB, HS, KS, WS, KS)
    pp = np.ascontiguousarray(x6.transpose(0, 2, 4, 3, 5, 1)).reshape(
        2, P, KS * KS, NB)
    # main chunks: pixel j in [0,768) -> row j%128, col (p, t=j//128, b);
    # chunks t < NT8 quantized to fp8-e4m3, the rest stay bf16
    e4 = ml_dtypes.float8_e4m3
    main = pp[:, :, :TM * 128].reshape(2, P, TM, 128, NB)
    x8h = np.ascontiguousarray(
        main[:, :, :NT8].transpose(0, 3, 1, 2, 4)).astype(e4)
    xmh = np.ascontiguousarray(main[:, :, NT8:].transpose(0, 3, 1, 2, 4))
    tl = pp[:, :, TM * 128:]                      # (2, P, 16, NB)

    # weight row j = f*100 + p (pixel-major k*28+l)
    w3 = np.asarray(weight, np.float32).reshape(F, P, KS * KS)
    wm = np.ascontiguousarray(
        w3[:, :, :TM * 128].reshape(F, P, TM, 128).transpose(3, 1, 2, 0)
    ).astype(bf16)                                # (128, P, TM, F)

    b2 = np.asarray(bias, np.float32).reshape(F, P)
    d3 = np.asarray(dec_w, np.float32).reshape(OUT, F, P)

    in_maps = []
    for core in range(NCORES):
        bh, q4 = core // 4, core % 4
        p0 = q4 * NP
        parts = []
        for g, qlo, qhi in SLICES:
            a, b = p0 + 4 * g + qlo, p0 + 4 * g + qhi
            parts.append(
                x8h[bh, :, a:b].reshape(128, -1).view(np.float32))
            parts.append(
                xmh[bh, :, a:b].reshape(128, -1).view(np.float32))
        xs = np.concatenate(parts, axis=1)
        ws = np.ascontiguousarray(wm[:, p0:p0 + NP]).reshape(
            128, NP * TM * F)
        xtl = np.zeros((128, NXB * NB), bf16)
        wtl = np.zeros((128, NP * F), bf16)
        bst = np.zeros((128, NG), np.float32)
        dst = np.zeros((128, NG * OUT), np.float32)
        for pl in range(NP):
            G, q = pl // 4, pl % 4
            r0 = 32 * ((pl % 8) // 2) + TAIL * (pl % 2)
            xtl[r0:r0 + TAIL, (pl // 8) * NB:(pl // 8 + 1) * NB] = \
                tl[bh, p0 + pl]
            wtl[r0:r0 + TAIL, pl * F:(pl + 1) * F] = \
                w3[:, p0 + pl, TM * 128:].T.astype(bf16)
            bst[32 * q:32 * q + F, G] = b2[:, p0 + pl]
            dst[32 * q:32 * q + F, G * OUT:(G + 1) * OUT] = d3[:, :, p0 + pl].T
        consts = np.concatenate(
            [ws.view(np.float32), wtl.view(np.float32), bst,
             dst.astype(bf16).view(np.float32), xtl.view(np.float32)],
            axis=1)
        in_maps.append({"consts": consts, "x": xs})
    return in_maps


def combine(results, dec_b):
    dec_b = np.asarray(dec_b, np.float32)
    out = np.zeros((B, OUT), np.float32)
    for bh in range(2):
        part = results[4 * bh]["out"].astype(np.float32)
        for q4 in range(1, 4):
            part = part + results[4 * bh + q4]["out"]
        out[bh * NB:(bh + 1) * NB] = part.T + dec_b
    return out


_cache = {}


def _get_nc():
    if "nc" not in _cache:
        nc = build_program()
        nc.finalize()
        _cache["nc"] = nc
    return _cache["nc"]


def _install_ntff_hook():
    """Provide the missing antenv.axon_hooks module so trace=True works
    under axon (replicates trn_boot._ntff_profile_via_ctypes)."""
    import contextlib
    import ctypes
    import types

    if "antenv.axon_hooks" in sys.modules:
        return
    so_path = "/opt/axon/libaxon_pjrt.so"
    holder = {}
    mod = types.ModuleType("antenv.axon_hooks")
    mod.set_axon_ntff_profile_hook = lambda h: holder.__setitem__("h", h)
    mod.get_axon_ntff_profile_hook = lambda: holder.get("h")
    sys.modules["antenv.axon_hooks"] = mod
    try:
        import antenv
        antenv.axon_hooks = mod
    except ImportError:
        pass

    lib = ctypes.CDLL(so_path)
    if not hasattr(lib, "axon_start_nrt_profile"):
        return
    lib.axon_start_nrt_profile.argtypes = [
        ctypes.POINTER(ctypes.c_int64), ctypes.c_size_t]
    lib.axon_start_nrt_profile.restype = ctypes.c_int64
    lib.axon_stop_nrt_profile.argtypes = [ctypes.c_char_p]
    lib.axon_stop_nrt_profile.restype = ctypes.c_int64

    @contextlib.contextmanager
    def _hook(output_dir, device_ids):
        import jax
        jax.devices()
        if device_ids:
            ids = (ctypes.c_int64 * len(device_ids))(*device_ids)
            rc = lib.axon_start_nrt_profile(ids, len(device_ids))
        else:
            rc = lib.axon_start_nrt_profile(None, 0)
        if rc != 0:
            raise RuntimeError(f"axon_start_nrt_profile rc={rc}")
        try:
            yield
        finally:
            n = lib.axon_stop_nrt_profile(str(output_dir).encode())
            print(f"profile: {n} file(s) written to {output_dir}")

    mod.set_axon_ntff_profile_hook(_hook)


def run(x, weight, bias, dec_w, dec_b, trace=False):
    from concourse import bass_utils
    from concourse.bass_utils import run_bass_kernel_spmd

    if trace:
        _install_ntff_hook()
        # artifact upload needs a bucket that doesn't exist here
        bass_utils.upload_artifacts = lambda tmpdir: tmpdir

    nc = _get_nc()
    in_maps = make_in_maps(x, weight, bias, dec_w)
    r = run_bass_kernel_spmd(nc, in_maps, list(range(NCORES)), trace=trace)
    return combine(r.results, dec_b), r


def kernel(x, weight, bias, dec_w, dec_b):
    out, _ = run(x, weight, bias, dec_w, dec_b, trace=False)
    return out


# revision 57
# speedup vs baseline: 1.3954x; 1.1715x over previous
"""Trainium2 Bass kernel for nn_LCN (locally-connected network).

Computation:
  x: (512, 1, 280, 280) -> non-overlapping 28x28 patches (10x10 grid, P=100)
  y[b, f, p] = sum_{k,l} x[b, 28ph+k, 28pw+l] * w[f*100+p, 0, k, l]
  y = relu(y + bias[f*100+p]);  out = y_flat @ dec_w.T + dec_b   (j = f*100 + p)

Strategy (v3): the problem is HBM-stream-bound, so minimize bytes moved and
keep the device program trivial:
  - Host casts x to bf16 (error budget 2e-2 >> bf16's ~5e-3; PE computed in
    bf16 anyway) -> halves the dominant x stream vs fp32.
  - Host pre-transposes x into contraction-major chunk layout so the PE
    does NO transposes / im2col: just matmuls, straight off the DMA stream.
  - DMA details that matter (measured): bf16-typed DMA runs at half the
    AXI beat width, so all streams are f32-typed and bitcast to bf16 for
    the PE; and only 128-partition-row DMAs align the 16 SDMA engines 1:1
    with the 16 SBUF AXI ports (112 rows -> pairwise port contention,
    16 vs 25 GB/s/engine). Hence each patch's 784 pixels are split as
    6 full K=128 chunks + a 16-pixel tail; tails of 4 patches pack into
    the 4 32-row strips of one column block and are applied by diagonal
    row+col-tiled K=16 matmuls.
  - Sharding: 2 batch halves (256 images) x 4 patch quarters (25 patches).
    Per core: x 9.8+0.5 MB + w 0.7 MB -> ~32us at ~343 GB/s/NC measured.
  - Mains: per patch 6 accumulating bf16 matmuls lhsT=w[128,16],
    rhs=x[128,256] -> y PSUM, 4 patches per PSUM tile at col offsets
    0/32/64/96 (tile_position -> concurrent col-tile streams), + 1 tail
    matmul lhsT=wt[16,16] at tile_position (32q,32q).
  - ACT: relu(y + bias) -> y_sb bf16; decoder matmul per group deferred by
    one group so the relu never stalls the PE; accumulated in PSUM.
  - The 1-patch group is processed first so the last DMA slice finishes
    into a wide 4-way-concurrent matmul tail.
Host sums the 4 patch-quarter partial decoder outputs and adds dec_b.
"""

import sys

import numpy as np

for _p in ("/opt/trn_rl_repo", "/opt/trn_rl_repo/concourse"):
    if _p not in sys.path:
        sys.path.insert(0, _p)

import concourse.bass as bass
import concourse.mybir as mybir
import concourse.tile as tile
from concourse import bacc

F32 = mybir.dt.float32
BF16 = mybir.dt.bfloat16
FP8 = mybir.dt.float8e3

# Problem constants
B, H, W = 512, 280, 280
KS = 28
HS = WS = 10
P = 100         # patches per image
F = 16
OUT = 10
NCORES = 8

# Sharding: 2 batch halves x 4 patch quarters
NB = 256        # images per core
NP = 25         # patches per core
TM = 6          # full K=128 chunks per patch (768 of 784 pixels)
TAIL = 16       # leftover pixels per patch (pixel 768..783)
NG = (NP + 3) // 4  # PSUM groups of 4 patches (last group: 1 patch)
NXB = (NP + 7) // 8  # x-tail column blocks (8 patch tails per 128-row block)
YB = 3          # y PSUM bufs

# Mixed precision: ALL of x is fp8-e3m4 (4 mantissa bits; range +-15.5
# covers x's max 5.4). Simulated end-to-end rel err 0.01394 vs the 2e-2
# gate (bf16-only: 0.0046, e4m3 on 256px: 0.0158); x stream drops from
# 6.55 to 4.92 MB/core. w stays bf16 (mixed-dtype matmuls are supported).
NT8 = TM
NTM = TM - NT8
SLW = NT8 * NB // 4 + NTM * NB // 2  # 384 f32 cols per patch in x_d

# processing order (1-patch group first) and DMA slices (last group split)
ORDER = [NG - 1] + list(range(NG - 1))
SLICES = []
for _g in ORDER:
    _npg = min(4, NP - 4 * _g)
    if _g == ORDER[-1]:
        SLICES += [(_g, 0, 2), (_g, 2, _npg)]
    else:
        SLICES += [(_g, 0, _npg)]


def build_program():
    nc = bacc.Bacc("TRN2")
    # All streams are f32-typed (bf16 data, bitcast on the PE side) and use
    # 128 partition rows -- both required for full DMA rate (see docstring).
    # Everything that isn't the main x stream (w, w-tail, bias, dec, x-tail)
    # is packed into ONE const tensor: separate small DMAs decay into
    # 128 descriptor-dominated packets that crawl behind the x stream and
    # stall the in-order PE queue at the first decoder matmul.
    # x tails are packed DENSE: patch pl's 16 tail pixels at rows
    # [32*((pl%8)//2) + 16*(pl%2)] of column block pl//8; the K=32 tail
    # matmul covers two patches' rows, with the other patch's 16 rows
    # zeroed in the w operand.
    CW = NP * TM * F // 2       # 1200  w main (f32 cols)
    CWT = NP * F // 2           # 200   w tail (K=32 halves, bf16 -> f32 cols)
    CB = NG                     # 7     bias (real f32)
    CD = NG * OUT // 2          # 35    dec (bf16 pairs)
    CXT = NXB * NB // 4         # 256   x tail (fp8-e3m4)
    CC = CW + CWT + CB + CD + CXT
    c_d = nc.dram_tensor("consts", [128, CC], F32, kind="ExternalInput")
    x_d = nc.dram_tensor("x", [128, NP * SLW], F32, kind="ExternalInput")
    o_d = nc.dram_tensor("out", [OUT, NB], F32, kind="ExternalOutput")

    with tile.TileContext(nc) as tc:
        with (
            tc.tile_pool(name="const", bufs=1) as constp,
            tc.tile_pool(name="xs", bufs=1) as xsp,
            tc.tile_pool(name="yps", bufs=YB, space="PSUM") as ypsp,
            tc.tile_pool(name="ops", bufs=1, space="PSUM") as opsp,
            tc.tile_pool(name="warm", bufs=1, space="PSUM") as warmp,
        ):
            c_sb = constp.tile([128, CC], F32)
            y_sb = constp.tile([128, NG * NB], BF16)
            zero_sb = constp.tile([128, NB], F32)
            out_sb = constp.tile([OUT, NB], F32)
            out_ps = opsp.tile([OUT, NB], F32)

            # consts first on the sync ring: one efficient wide DMA
            nc.sync.dma_start(out=c_sb[:], in_=c_d[:])
            nc.gpsimd.memset(zero_sb[:], 0.0)

            # HAM warm-up: bf16 matmuls on varied (iota) data into scratch
            # PSUM while the PE would otherwise idle waiting for the first x
            # slice. Unthrottles the PE clock gate (1.2 -> 2.4 GHz). fp32 or
            # all-zero matmuls do NOT register as PE activity.
            iota_sb = constp.tile([128, NB], mybir.dt.int32)
            nc.gpsimd.iota(iota_sb[:], [[1, NB]], channel_multiplier=77)
            iota_bf = iota_sb[:].bitcast(BF16)
            warm_ps = warmp.tile([128, 2 * NB], F32)

            def warm(n):
                for _ in range(n):
                    nc.tensor.matmul(
                        warm_ps[0:F, :], iota_bf[:, 0:F], iota_bf[:],
                        start=True, stop=True)

            warm(10)

            # x slices in consumption order; each slice = [fp8 block | bf16
            # block] for its patches, one DMA per slice
            x_tiles = {}
            off = 0
            for g, qlo, qhi in SLICES:
                n = qhi - qlo
                t = xsp.tile([128, n * SLW], F32, name=f"xg{g}_{qlo}")
                nc.sync.dma_start(out=t[:], in_=x_d[:, off:off + n * SLW])
                x_tiles[(g, qlo)] = t
                off += n * SLW
            w_bf = c_sb[:, 0:CW].bitcast(BF16)
            wt_bf = c_sb[:, CW:CW + CWT].bitcast(BF16)
            bias_sb = c_sb[:, CW + CWT:CW + CWT + CB]
            dec_bf = c_sb[:, CW + CWT + CB:CW + CWT + CB + CD].bitcast(BF16)
            xt_bf = c_sb[:, CW + CWT + CB + CD:CC].bitcast(FP8)

            def emit_dec(g, first, last):
                nc.tensor.matmul(
                    out_ps[:],
                    dec_bf[:, g * OUT:(g + 1) * OUT],
                    y_sb[:, g * NB:(g + 1) * NB],
                    start=first,
                    stop=last,
                )

            prev_g = None
            for idx, g in enumerate(ORDER):
                npg = min(4, NP - 4 * g)
                yt = ypsp.tile([128, NB], F32, name="y_ps")
                if idx < YB:
                    # clear stale/NaN PSUM so the gap rows are finite zeros
                    nc.vector.tensor_copy(yt[:], zero_sb[:])
                for g2, qlo, qhi in [s for s in SLICES if s[0] == g]:
                    n = qhi - qlo
                    xg = x_tiles[(g, qlo)]
                    x8 = xg[:, 0:n * NT8 * NB // 4].bitcast(FP8)
                    xm = (xg[:, n * NT8 * NB // 4:n * SLW].bitcast(BF16)
                          if NTM else None)
                    for t in range(TM):
                        for q in range(qlo, qhi):
                            pl = 4 * g + q
                            if t < NT8:
                                rhs = x8[:, ((q - qlo) * NT8 + t) * NB:
                                         ((q - qlo) * NT8 + t + 1) * NB]
                            else:
                                t2 = t - NT8
                                rhs = xm[:, ((q - qlo) * NTM + t2) * NB:
                                         ((q - qlo) * NTM + t2 + 1) * NB]
                            nc.tensor.matmul(
                                yt[32 * q:32 * q + F, :],
                                w_bf[:, (pl * TM + t) * F:
                                     (pl * TM + t + 1) * F],
                                rhs,
                                start=(t == 0),
                                stop=False,
                                tile_position=(0, 32 * q),
                            )
                    for q in range(qlo, qhi):
                        # 16-pixel tail: K=32 matmul over the dense-packed
                        # tail strip (covers this patch + its pair patch,
                        # whose rows are zero in the w operand)
                        pl = 4 * g + q
                        a2, blk = (pl % 8) // 2, pl // 8
                        nc.tensor.matmul(
                            yt[32 * q:32 * q + F, :],
                            wt_bf[32 * a2:32 * a2 + 32, pl * F:(pl + 1) * F],
                            xt_bf[32 * a2:32 * a2 + 32, blk * NB:(blk + 1) * NB],
                            start=False,
                            stop=True,
                            tile_position=(32 * a2, 32 * q),
                        )
                    if g == ORDER[-1] and qhi < npg:
                        # keep the PE clock warm across the inter-sub-slice
                        # DMA wait before the final tail
                        warm(8)
                if g != ORDER[-1]:
                    nc.scalar.activation(
                        out=y_sb[:, g * NB:(g + 1) * NB],
                        in_=yt[:],
                        func=mybir.ActivationFunctionType.Relu,
                        bias=bias_sb[:, g:g + 1],
                    )
                    # decoder deferred one group so its y_sb dependency (the
                    # relu above) never stalls the PE behind independent mains
                    if prev_g is not None:
                        emit_dec(prev_g, prev_g == ORDER[0], False)
                    prev_g = g
                    if idx >= 1:
                        # fillers across the slice-wait gaps so the PE clock
                        # stays warm through the kernel tail
                        warm(8)
                else:
                    # final group: flush the deferred decoder, then pipeline
                    # relu -> dec -> psum-copy in column halves to shorten
                    # the serial dependency chain at the kernel tail
                    if prev_g is not None:
                        emit_dec(prev_g, prev_g == ORDER[0], False)
                    for h in range(2):
                        cols = slice(g * NB + h * 128, g * NB + h * 128 + 128)
                        nc.scalar.activation(
                            out=y_sb[:, cols],
                            in_=yt[:, h * 128:h * 128 + 128],
                            func=mybir.ActivationFunctionType.Relu,
                            bias=bias_sb[:, g:g + 1],
                        )
                        nc.tensor.matmul(
                            out_ps[:, h * 128:h * 128 + 128],
                            dec_bf[:, g * OUT:(g + 1) * OUT],
                            y_sb[:, cols],
                            start=False,
                            stop=True,
                        )
                        nc.vector.tensor_copy(
                            out_sb[:, h * 128:h * 128 + 128],
                            out_ps[:, h * 128:h * 128 + 128])

            nc.sync.dma_start(out=o_d[:], in_=out_sb[:])

    return nc


def make_in_maps(x, weight, bias, dec_w):
    import ml_dtypes
    bf16 = ml_dtypes.bfloat16

    x = np.asarray(x, np.float32).reshape(B, H, W)
    xb = x.astype(bf16)
    # (bh, b, hs, k, ws, l) -> (bh, hs, ws, k, l, b): per-patch pixel-major
    x6 = xb.reshape(2, NB, HS, KS, WS, KS)
    pp = np.ascontiguousarray(x6.transpose(0, 2, 4, 3, 5, 1)).reshape(
        2, P, KS * KS, NB)
    # main chunks: pixel j in [0,768) -> row j%128, col (p, t=j//128, b);
    # chunks t < NT8 quantized to fp8-e3m4, the rest stay bf16
    e3 = ml_dtypes.float8_e3m4
    main = pp[:, :, :TM * 128].reshape(2, P, TM, 128, NB)
    x8h = np.ascontiguousarray(
        main[:, :, :NT8].transpose(0, 3, 1, 2, 4)).astype(e3)
    xmh = np.ascontiguousarray(main[:, :, NT8:].transpose(0, 3, 1, 2, 4))
    tl = pp[:, :, TM * 128:].astype(e3)           # (2, P, 16, NB) fp8 tail

    # weight row j = f*100 + p (pixel-major k*28+l)
    w3 = np.asarray(weight, np.float32).reshape(F, P, KS * KS)
    wm = np.ascontiguousarray(
        w3[:, :, :TM * 128].reshape(F, P, TM, 128).transpose(3, 1, 2, 0)
    ).astype(bf16)                                # (128, P, TM, F)

    b2 = np.asarray(bias, np.float32).reshape(F, P)
    d3 = np.asarray(dec_w, np.float32).reshape(OUT, F, P)

    in_maps = []
    for core in range(NCORES):
        bh, q4 = core // 4, core % 4
        p0 = q4 * NP
        parts = []
        for g, qlo, qhi in SLICES:
            a, b = p0 + 4 * g + qlo, p0 + 4 * g + qhi
            parts.append(
                x8h[bh, :, a:b].reshape(128, -1).view(np.float32))
            if NTM:
                parts.append(
                    xmh[bh, :, a:b].reshape(128, -1).view(np.float32))
        xs = np.concatenate(parts, axis=1)
        ws = np.ascontiguousarray(wm[:, p0:p0 + NP]).reshape(
            128, NP * TM * F)
        xtl = np.zeros((128, NXB * NB), e3)
        wtl = np.zeros((128, NP * F), bf16)
        bst = np.zeros((128, NG), np.float32)
        dst = np.zeros((128, NG * OUT), np.float32)
        for pl in range(NP):
            G, q = pl // 4, pl % 4
            r0 = 32 * ((pl % 8) // 2) + TAIL * (pl % 2)
            xtl[r0:r0 + TAIL, (pl // 8) * NB:(pl // 8 + 1) * NB] = \
                tl[bh, p0 + pl]
            wtl[r0:r0 + TAIL, pl * F:(pl + 1) * F] = \
                w3[:, p0 + pl, TM * 128:].T.astype(bf16)
            bst[32 * q:32 * q + F, G] = b2[:, p0 + pl]
            dst[32 * q:32 * q + F, G * OUT:(G + 1) * OUT] = d3[:, :, p0 + pl].T
        consts = np.concatenate(
            [ws.view(np.float32), wtl.view(np.float32), bst,
             dst.astype(bf16).view(np.float32), xtl.view(np.float32)],
            axis=1)
        in_maps.append({"consts": consts, "x": xs})
    return in_maps


def combine(results, dec_b):
    dec_b = np.asarray(dec_b, np.float32)
    out = np.zeros((B, OUT), np.float32)
    for bh in range(2):
        part = results[4 * bh]["out"].astype(np.float32)
        for q4 in range(1, 4):
            part = part + results[4 * bh + q4]["out"]
        out[bh * NB:(bh + 1) * NB] = part.T + dec_b
    return out


_cache = {}


def _get_nc():
    if "nc" not in _cache:
        nc = build_program()
        nc.finalize()
        _cache["nc"] = nc
    return _cache["nc"]


def _install_ntff_hook():
    """Provide the missing antenv.axon_hooks module so trace=True works
    under axon (replicates trn_boot._ntff_profile_via_ctypes)."""
    import contextlib
    import ctypes
    import types

    if "antenv.axon_hooks" in sys.modules:
        return
    so_path = "/opt/axon/libaxon_pjrt.so"
    holder = {}
    mod = types.ModuleType("antenv.axon_hooks")
    mod.set_axon_ntff_profile_hook = lambda h: holder.__setitem__("h", h)
    mod.get_axon_ntff_profile_hook = lambda: holder.get("h")
    sys.modules["antenv.axon_hooks"] = mod
    try:
        import antenv
        antenv.axon_hooks = mod
    except ImportError:
        pass

    lib = ctypes.CDLL(so_path)
    if not hasattr(lib, "axon_start_nrt_profile"):
        return
    lib.axon_start_nrt_profile.argtypes = [
        ctypes.POINTER(ctypes.c_int64), ctypes.c_size_t]
    lib.axon_start_nrt_profile.restype = ctypes.c_int64
    lib.axon_stop_nrt_profile.argtypes = [ctypes.c_char_p]
    lib.axon_stop_nrt_profile.restype = ctypes.c_int64

    @contextlib.contextmanager
    def _hook(output_dir, device_ids):
        import jax
        jax.devices()
        if device_ids:
            ids = (ctypes.c_int64 * len(device_ids))(*device_ids)
            rc = lib.axon_start_nrt_profile(ids, len(device_ids))
        else:
            rc = lib.axon_start_nrt_profile(None, 0)
        if rc != 0:
            raise RuntimeError(f"axon_start_nrt_profile rc={rc}")
        try:
            yield
        finally:
            n = lib.axon_stop_nrt_profile(str(output_dir).encode())
            print(f"profile: {n} file(s) written to {output_dir}")

    mod.set_axon_ntff_profile_hook(_hook)


def run(x, weight, bias, dec_w, dec_b, trace=False):
    from concourse import bass_utils
    from concourse.bass_utils import run_bass_kernel_spmd

    if trace:
        _install_ntff_hook()
        # artifact upload needs a bucket that doesn't exist here
        bass_utils.upload_artifacts = lambda tmpdir: tmpdir

    nc = _get_nc()
    in_maps = make_in_maps(x, weight, bias, dec_w)
    r = run_bass_kernel_spmd(nc, in_maps, list(range(NCORES)), trace=trace)
    return combine(r.results, dec_b), r


def kernel(x, weight, bias, dec_w, dec_b):
    out, _ = run(x, weight, bias, dec_w, dec_b, trace=False)
    return out


# revision 59
# speedup vs baseline: 1.4663x; 1.0508x over previous
"""Trainium2 Bass kernel for nn_LCN (locally-connected network).

Computation:
  x: (512, 1, 280, 280) -> non-overlapping 28x28 patches (10x10 grid, P=100)
  y[b, f, p] = sum_{k,l} x[b, 28ph+k, 28pw+l] * w[f*100+p, 0, k, l]
  y = relu(y + bias[f*100+p]);  out = y_flat @ dec_w.T + dec_b   (j = f*100 + p)

Strategy (v3): the problem is HBM-stream-bound, so minimize bytes moved and
keep the device program trivial:
  - Host casts x to bf16 (error budget 2e-2 >> bf16's ~5e-3; PE computed in
    bf16 anyway) -> halves the dominant x stream vs fp32.
  - Host pre-transposes x into contraction-major chunk layout so the PE
    does NO transposes / im2col: just matmuls, straight off the DMA stream.
  - DMA details that matter (measured): bf16-typed DMA runs at half the
    AXI beat width, so all streams are f32-typed and bitcast to bf16 for
    the PE; and only 128-partition-row DMAs align the 16 SDMA engines 1:1
    with the 16 SBUF AXI ports (112 rows -> pairwise port contention,
    16 vs 25 GB/s/engine). Hence each patch's 784 pixels are split as
    6 full K=128 chunks + a 16-pixel tail; tails of 4 patches pack into
    the 4 32-row strips of one column block and are applied by diagonal
    row+col-tiled K=16 matmuls.
  - Sharding: 2 batch halves (256 images) x 4 patch quarters (25 patches).
    Per core: x 9.8+0.5 MB + w 0.7 MB -> ~32us at ~343 GB/s/NC measured.
  - Mains: per patch 6 accumulating bf16 matmuls lhsT=w[128,16],
    rhs=x[128,256] -> y PSUM, 4 patches per PSUM tile at col offsets
    0/32/64/96 (tile_position -> concurrent col-tile streams), + 1 tail
    matmul lhsT=wt[16,16] at tile_position (32q,32q).
  - ACT: relu(y + bias) -> y_sb bf16; decoder matmul per group deferred by
    one group so the relu never stalls the PE; accumulated in PSUM.
  - The 1-patch group is processed first so the last DMA slice finishes
    into a wide 4-way-concurrent matmul tail.
Host sums the 4 patch-quarter partial decoder outputs and adds dec_b.
"""

import sys

import numpy as np

for _p in ("/opt/trn_rl_repo", "/opt/trn_rl_repo/concourse"):
    if _p not in sys.path:
        sys.path.insert(0, _p)

import concourse.bass as bass
import concourse.mybir as mybir
import concourse.tile as tile
from concourse import bacc

F32 = mybir.dt.float32
BF16 = mybir.dt.bfloat16
FP8 = mybir.dt.float8e3

# Problem constants
B, H, W = 512, 280, 280
KS = 28
HS = WS = 10
P = 100         # patches per image
F = 16
OUT = 10
NCORES = 8

# Sharding: 2 batch halves x 4 patch quarters
NB = 256        # images per core
NP = 25         # patches per core
TM = 6          # full K=128 chunks per patch (768 of 784 pixels)
TAIL = 16       # leftover pixels per patch (pixel 768..783)
NG = (NP + 3) // 4  # PSUM groups of 4 patches (last group: 1 patch)
NXB = (NP + 7) // 8  # x-tail column blocks (8 patch tails per 128-row block)
YB = 3          # y PSUM bufs

# Mixed precision: ALL of x is fp8-e3m4 (4 mantissa bits; range +-15.5
# covers x's max 5.4). Simulated end-to-end rel err 0.01394 vs the 2e-2
# gate (bf16-only: 0.0046, e4m3 on 256px: 0.0158); x stream drops from
# 6.55 to 4.92 MB/core. w stays bf16 (mixed-dtype matmuls are supported).
NT8 = TM
NTM = TM - NT8
SLW = NT8 * NB // 4 + NTM * NB // 2  # 384 f32 cols per patch in x_d

# processing order (1-patch group first) and DMA slices (last group split)
ORDER = [NG - 1] + list(range(NG - 1))
SLICES = []
for _g in ORDER:
    _npg = min(4, NP - 4 * _g)
    if _g == ORDER[-1]:
        SLICES += [(_g, 0, 2), (_g, 2, _npg)]
    else:
        SLICES += [(_g, 0, _npg)]


def build_program():
    nc = bacc.Bacc("TRN2")
    # All streams are f32-typed (bf16 data, bitcast on the PE side) and use
    # 128 partition rows -- both required for full DMA rate (see docstring).
    # Everything that isn't the main x stream (w, w-tail, bias, dec, x-tail)
    # is packed into ONE const tensor: separate small DMAs decay into
    # 128 descriptor-dominated packets that crawl behind the x stream and
    # stall the in-order PE queue at the first decoder matmul.
    # x tails are packed DENSE: patch pl's 16 tail pixels at rows
    # [32*((pl%8)//2) + 16*(pl%2)] of column block pl//8; the K=32 tail
    # matmul covers two patches' rows, with the other patch's 16 rows
    # zeroed in the w operand.
    CW = NP * TM * F // 2       # 1200  w main (f32 cols)
    CWT = NP * F // 2           # 200   w tail (K=32 halves, bf16 -> f32 cols)
    CB = NG                     # 7     bias (real f32)
    CD = NG * OUT // 2          # 35    dec (bf16 pairs)
    CXT = NXB * NB // 4         # 256   x tail (fp8-e3m4)
    CC = CW + CWT + CB + CD + CXT
    c_d = nc.dram_tensor("consts", [128, CC], F32, kind="ExternalInput")
    x_d = nc.dram_tensor("x", [128, NP * SLW], F32, kind="ExternalInput")
    o_d = nc.dram_tensor("out", [OUT, NB], F32, kind="ExternalOutput")

    with tile.TileContext(nc) as tc:
        with (
            tc.tile_pool(name="const", bufs=1) as constp,
            tc.tile_pool(name="xs", bufs=1) as xsp,
            tc.tile_pool(name="yps", bufs=YB, space="PSUM") as ypsp,
            tc.tile_pool(name="ops", bufs=1, space="PSUM") as opsp,
            tc.tile_pool(name="warm", bufs=1, space="PSUM") as warmp,
        ):
            c_sb = constp.tile([128, CC], F32)
            y_sb = constp.tile([128, NG * NB], BF16)
            zero_sb = constp.tile([128, NB], F32)
            out_sb = constp.tile([OUT, NB], F32)
            out_ps = opsp.tile([OUT, NB], F32)

            # consts first on the sync ring: one efficient wide DMA
            nc.sync.dma_start(out=c_sb[:], in_=c_d[:])
            nc.gpsimd.memset(zero_sb[:], 0.0)

            # HAM warm-up: bf16 matmuls on varied (iota) data into scratch
            # PSUM while the PE would otherwise idle waiting for the first x
            # slice. Unthrottles the PE clock gate (1.2 -> 2.4 GHz). fp32 or
            # all-zero matmuls do NOT register as PE activity.
            iota_sb = constp.tile([128, NB], mybir.dt.int32)
            nc.gpsimd.iota(iota_sb[:], [[1, NB]], channel_multiplier=77)
            iota_bf = iota_sb[:].bitcast(BF16)
            warm_ps = warmp.tile([128, 2 * NB], F32)

            def warm(n):
                for _ in range(n):
                    nc.tensor.matmul(
                        warm_ps[0:F, :], iota_bf[:, 0:F], iota_bf[:],
                        start=True, stop=True)

            warm(10)

            # x slices in consumption order; each slice = [fp8 block | bf16
            # block] for its patches, one DMA per slice
            x_tiles = {}
            off = 0
            for g, qlo, qhi in SLICES:
                n = qhi - qlo
                t = xsp.tile([128, n * SLW], F32, name=f"xg{g}_{qlo}")
                nc.sync.dma_start(out=t[:], in_=x_d[:, off:off + n * SLW])
                x_tiles[(g, qlo)] = t
                off += n * SLW
            w_bf = c_sb[:, 0:CW].bitcast(BF16)
            wt_bf = c_sb[:, CW:CW + CWT].bitcast(BF16)
            bias_sb = c_sb[:, CW + CWT:CW + CWT + CB]
            dec_bf = c_sb[:, CW + CWT + CB:CW + CWT + CB + CD].bitcast(BF16)
            xt_bf = c_sb[:, CW + CWT + CB + CD:CC].bitcast(FP8)

            def emit_dec(g, first, last):
                nc.tensor.matmul(
                    out_ps[:],
                    dec_bf[:, g * OUT:(g + 1) * OUT],
                    y_sb[:, g * NB:(g + 1) * NB],
                    start=first,
                    stop=last,
                )

            prev_g = None
            for idx, g in enumerate(ORDER):
                npg = min(4, NP - 4 * g)
                yt = ypsp.tile([128, NB], F32, name="y_ps")
                if idx < YB:
                    # clear stale/NaN PSUM so the gap rows are finite zeros
                    nc.vector.tensor_copy(yt[:], zero_sb[:])
                for g2, qlo, qhi in [s for s in SLICES if s[0] == g]:
                    n = qhi - qlo
                    xg = x_tiles[(g, qlo)]
                    x8 = xg[:, 0:n * NT8 * NB // 4].bitcast(FP8)
                    xm = (xg[:, n * NT8 * NB // 4:n * SLW].bitcast(BF16)
                          if NTM else None)
                    for t in range(TM):
                        for q in range(qlo, qhi):
                            pl = 4 * g + q
                            if t < NT8:
                                rhs = x8[:, ((q - qlo) * NT8 + t) * NB:
                                         ((q - qlo) * NT8 + t + 1) * NB]
                            else:
                                t2 = t - NT8
                                rhs = xm[:, ((q - qlo) * NTM + t2) * NB:
                                         ((q - qlo) * NTM + t2 + 1) * NB]
                            nc.tensor.matmul(
                                yt[32 * q:32 * q + F, :],
                                w_bf[:, (pl * TM + t) * F:
                                     (pl * TM + t + 1) * F],
                                rhs,
                                start=(t == 0),
                                stop=False,
                                tile_position=(0, 32 * q),
                            )
                    for q in range(qlo, qhi):
                        # 16-pixel tail: K=32 matmul over the dense-packed
                        # tail strip (covers this patch + its pair patch,
                        # whose rows are zero in the w operand)
                        pl = 4 * g + q
                        a2, blk = (pl % 8) // 2, pl // 8
                        nc.tensor.matmul(
                            yt[32 * q:32 * q + F, :],
                            wt_bf[32 * a2:32 * a2 + 32, pl * F:(pl + 1) * F],
                            xt_bf[32 * a2:32 * a2 + 32, blk * NB:(blk + 1) * NB],
                            start=False,
                            stop=True,
                            tile_position=(32 * a2, 32 * q),
                        )

                if g != ORDER[-1]:
                    nc.scalar.activation(
                        out=y_sb[:, g * NB:(g + 1) * NB],
                        in_=yt[:],
                        func=mybir.ActivationFunctionType.Relu,
                        bias=bias_sb[:, g:g + 1],
                    )
                    # decoder deferred one group so its y_sb dependency (the
                    # relu above) never stalls the PE behind independent mains
                    if prev_g is not None:
                        emit_dec(prev_g, prev_g == ORDER[0], False)
                    prev_g = g
                    # (no per-group fillers: with the fp8 stream the slice
                    # cadence is ~2.3us and real matmul work keeps the PE
                    # clock warm; fillers would serialize ahead of real MMs)
                else:
                    # final group: flush the deferred decoder, then pipeline
                    # relu -> dec -> psum-copy in column halves to shorten
                    # the serial dependency chain at the kernel tail
                    if prev_g is not None:
                        emit_dec(prev_g, prev_g == ORDER[0], False)
                    for h in range(2):
                        cols = slice(g * NB + h * 128, g * NB + h * 128 + 128)
                        nc.scalar.activation(
                            out=y_sb[:, cols],
                            in_=yt[:, h * 128:h * 128 + 128],
                            func=mybir.ActivationFunctionType.Relu,
                            bias=bias_sb[:, g:g + 1],
                        )
                        nc.tensor.matmul(
                            out_ps[:, h * 128:h * 128 + 128],
                            dec_bf[:, g * OUT:(g + 1) * OUT],
                            y_sb[:, cols],
                            start=False,
                            stop=True,
                        )
                        nc.vector.tensor_copy(
                            out_sb[:, h * 128:h * 128 + 128],
                            out_ps[:, h * 128:h * 128 + 128])

            nc.sync.dma_start(out=o_d[:], in_=out_sb[:])

    return nc


def make_in_maps(x, weight, bias, dec_w):
    import ml_dtypes
    bf16 = ml_dtypes.bfloat16

    x = np.asarray(x, np.float32).reshape(B, H, W)
    xb = x.astype(bf16)
    # (bh, b, hs, k, ws, l) -> (bh, hs, ws, k, l, b): per-patch pixel-major
    x6 = xb.reshape(2, NB, HS, KS, WS, KS)
    pp = np.ascontiguousarray(x6.transpose(0, 2, 4, 3, 5, 1)).reshape(
        2, P, KS * KS, NB)
    # main chunks: pixel j in [0,768) -> row j%128, col (p, t=j//128, b);
    # chunks t < NT8 quantized to fp8-e3m4, the rest stay bf16
    e3 = ml_dtypes.float8_e3m4
    main = pp[:, :, :TM * 128].reshape(2, P, TM, 128, NB)
    x8h = np.ascontiguousarray(
        main[:, :, :NT8].transpose(0, 3, 1, 2, 4)).astype(e3)
    xmh = np.ascontiguousarray(main[:, :, NT8:].transpose(0, 3, 1, 2, 4))
    tl = pp[:, :, TM * 128:].astype(e3)           # (2, P, 16, NB) fp8 tail

    # weight row j = f*100 + p (pixel-major k*28+l)
    w3 = np.asarray(weight, np.float32).reshape(F, P, KS * KS)
    wm = np.ascontiguousarray(
        w3[:, :, :TM * 128].reshape(F, P, TM, 128).transpose(3, 1, 2, 0)
    ).astype(bf16)                                # (128, P, TM, F)

    b2 = np.asarray(bias, np.float32).reshape(F, P)
    d3 = np.asarray(dec_w, np.float32).reshape(OUT, F, P)

    in_maps = []
    for core in range(NCORES):
        bh, q4 = core // 4, core % 4
        p0 = q4 * NP
        parts = []
        for g, qlo, qhi in SLICES:
            a, b = p0 + 4 * g + qlo, p0 + 4 * g + qhi
            parts.append(
                x8h[bh, :, a:b].reshape(128, -1).view(np.float32))
            if NTM:
                parts.append(
                    xmh[bh, :, a:b].reshape(128, -1).view(np.float32))
        xs = np.concatenate(parts, axis=1)
        ws = np.ascontiguousarray(wm[:, p0:p0 + NP]).reshape(
            128, NP * TM * F)
        xtl = np.zeros((128, NXB * NB), e3)
        wtl = np.zeros((128, NP * F), bf16)
        bst = np.zeros((128, NG), np.float32)
        dst = np.zeros((128, NG * OUT), np.float32)
        for pl in range(NP):
            G, q = pl // 4, pl % 4
            r0 = 32 * ((pl % 8) // 2) + TAIL * (pl % 2)
            xtl[r0:r0 + TAIL, (pl // 8) * NB:(pl // 8 + 1) * NB] = \
                tl[bh, p0 + pl]
            wtl[r0:r0 + TAIL, pl * F:(pl + 1) * F] = \
                w3[:, p0 + pl, TM * 128:].T.astype(bf16)
            bst[32 * q:32 * q + F, G] = b2[:, p0 + pl]
            dst[32 * q:32 * q + F, G * OUT:(G + 1) * OUT] = d3[:, :, p0 + pl].T
        consts = np.concatenate(
            [ws.view(np.float32), wtl.view(np.float32), bst,
             dst.astype(bf16).view(np.float32), xtl.view(np.float32)],
            axis=1)
        in_maps.append({"consts": consts, "x": xs})
    return in_maps


def combine(results, dec_b):
    dec_b = np.asarray(dec_b, np.float32)
    out = np.zeros((B, OUT), np.float32)
    for bh in range(2):
        part = results[4 * bh]["out"].astype(np.float32)
        for q4 in range(1, 4):
            part = part + results[4 * bh + q4]["out"]
        out[bh * NB:(bh + 1) * NB] = part.T + dec_b
    return out


_cache = {}


def _get_nc():
    if "nc" not in _cache:
        nc = build_program()
        nc.finalize()
        _cache["nc"] = nc
    return _cache["nc"]


def _install_ntff_hook():
    """Provide the missing antenv.axon_hooks module so trace=True works
    under axon (replicates trn_boot._ntff_profile_via_ctypes)."""
    import contextlib
    import ctypes
    import types

    if "antenv.axon_hooks" in sys.modules:
        return
    so_path = "/opt/axon/libaxon_pjrt.so"
    holder = {}
    mod = types.ModuleType("antenv.axon_hooks")
    mod.set_axon_ntff_profile_hook = lambda h: holder.__setitem__("h", h)
    mod.get_axon_ntff_profile_hook = lambda: holder.get("h")
    sys.modules["antenv.axon_hooks"] = mod
    try:
        import antenv
        antenv.axon_hooks = mod
    except ImportError:
        pass

    lib = ctypes.CDLL(so_path)
    if not hasattr(lib, "axon_start_nrt_profile"):
        return
    lib.axon_start_nrt_profile.argtypes = [
        ctypes.POINTER(ctypes.c_int64), ctypes.c_size_t]
    lib.axon_start_nrt_profile.restype = ctypes.c_int64
    lib.axon_stop_nrt_profile.argtypes = [ctypes.c_char_p]
    lib.axon_stop_nrt_profile.restype = ctypes.c_int64

    @contextlib.contextmanager
    def _hook(output_dir, device_ids):
        import jax
        jax.devices()
        if device_ids:
            ids = (ctypes.c_int64 * len(device_ids))(*device_ids)
            rc = lib.axon_start_nrt_profile(ids, len(device_ids))
        else:
            rc = lib.axon_start_nrt_profile(None, 0)
        if rc != 0:
            raise RuntimeError(f"axon_start_nrt_profile rc={rc}")
        try:
            yield
        finally:
            n = lib.axon_stop_nrt_profile(str(output_dir).encode())
            print(f"profile: {n} file(s) written to {output_dir}")

    mod.set_axon_ntff_profile_hook(_hook)


def run(x, weight, bias, dec_w, dec_b, trace=False):
    from concourse import bass_utils
    from concourse.bass_utils import run_bass_kernel_spmd

    if trace:
        _install_ntff_hook()
        # artifact upload needs a bucket that doesn't exist here
        bass_utils.upload_artifacts = lambda tmpdir: tmpdir

    nc = _get_nc()
    in_maps = make_in_maps(x, weight, bias, dec_w)
    r = run_bass_kernel_spmd(nc, in_maps, list(range(NCORES)), trace=trace)
    return combine(r.results, dec_b), r


def kernel(x, weight, bias, dec_w, dec_b):
    out, _ = run(x, weight, bias, dec_w, dec_b, trace=False)
    return out


# revision 61
# speedup vs baseline: 1.5020x; 1.0244x over previous
"""Trainium2 Bass kernel for nn_LCN (locally-connected network).

Computation:
  x: (512, 1, 280, 280) -> non-overlapping 28x28 patches (10x10 grid, P=100)
  y[b, f, p] = sum_{k,l} x[b, 28ph+k, 28pw+l] * w[f*100+p, 0, k, l]
  y = relu(y + bias[f*100+p]);  out = y_flat @ dec_w.T + dec_b   (j = f*100 + p)

Strategy (v3): the problem is HBM-stream-bound, so minimize bytes moved and
keep the device program trivial:
  - Host casts x to bf16 (error budget 2e-2 >> bf16's ~5e-3; PE computed in
    bf16 anyway) -> halves the dominant x stream vs fp32.
  - Host pre-transposes x into contraction-major chunk layout so the PE
    does NO transposes / im2col: just matmuls, straight off the DMA stream.
  - DMA details that matter (measured): bf16-typed DMA runs at half the
    AXI beat width, so all streams are f32-typed and bitcast to bf16 for
    the PE; and only 128-partition-row DMAs align the 16 SDMA engines 1:1
    with the 16 SBUF AXI ports (112 rows -> pairwise port contention,
    16 vs 25 GB/s/engine). Hence each patch's 784 pixels are split as
    6 full K=128 chunks + a 16-pixel tail; tails of 4 patches pack into
    the 4 32-row strips of one column block and are applied by diagonal
    row+col-tiled K=16 matmuls.
  - Sharding: 2 batch halves (256 images) x 4 patch quarters (25 patches).
    Per core: x 9.8+0.5 MB + w 0.7 MB -> ~32us at ~343 GB/s/NC measured.
  - Mains: per patch 6 accumulating bf16 matmuls lhsT=w[128,16],
    rhs=x[128,256] -> y PSUM, 4 patches per PSUM tile at col offsets
    0/32/64/96 (tile_position -> concurrent col-tile streams), + 1 tail
    matmul lhsT=wt[16,16] at tile_position (32q,32q).
  - ACT: relu(y + bias) -> y_sb bf16; decoder matmul per group deferred by
    one group so the relu never stalls the PE; accumulated in PSUM.
  - The 1-patch group is processed first so the last DMA slice finishes
    into a wide 4-way-concurrent matmul tail.
Host sums the 4 patch-quarter partial decoder outputs and adds dec_b.
"""

import sys

import numpy as np

for _p in ("/opt/trn_rl_repo", "/opt/trn_rl_repo/concourse"):
    if _p not in sys.path:
        sys.path.insert(0, _p)

import concourse.bass as bass
import concourse.mybir as mybir
import concourse.tile as tile
from concourse import bacc

F32 = mybir.dt.float32
BF16 = mybir.dt.bfloat16
FP8 = mybir.dt.float8e3

# Problem constants
B, H, W = 512, 280, 280
KS = 28
HS = WS = 10
P = 100         # patches per image
F = 16
OUT = 10
NCORES = 8

# Sharding: 2 batch halves x 4 patch quarters
NB = 256        # images per core
NP = 25         # patches per core
TM = 6          # full K=128 chunks per patch (768 of 784 pixels)
TAIL = 16       # leftover pixels per patch (pixel 768..783)
NG = (NP + 3) // 4  # PSUM groups of 4 patches (last group: 1 patch)
NXB = (NP + 7) // 8  # x-tail column blocks (8 patch tails per 128-row block)
YB = 3          # y PSUM bufs

# Mixed precision: ALL of x is fp8-e3m4 (4 mantissa bits; range +-15.5
# covers x's max 5.4). Simulated end-to-end rel err 0.01394 vs the 2e-2
# gate (bf16-only: 0.0046, e4m3 on 256px: 0.0158); x stream drops from
# 6.55 to 4.92 MB/core. w stays bf16 (mixed-dtype matmuls are supported).
NT8 = TM
NTM = TM - NT8
SLW = NT8 * NB // 4 + NTM * NB // 2  # 384 f32 cols per patch in x_d

# processing order (1-patch group first) and DMA slices (last group split)
ORDER = [NG - 1] + list(range(NG - 1))
SLICES = []
for _g in ORDER:
    _npg = min(4, NP - 4 * _g)
    if _g == ORDER[-1]:
        SLICES += [(_g, 0, 2), (_g, 2, _npg)]
    else:
        SLICES += [(_g, 0, _npg)]


def build_program():
    nc = bacc.Bacc("TRN2")
    # All streams are f32-typed (bf16 data, bitcast on the PE side) and use
    # 128 partition rows -- both required for full DMA rate (see docstring).
    # Everything that isn't the main x stream (w, w-tail, bias, dec, x-tail)
    # is packed into ONE const tensor: separate small DMAs decay into
    # 128 descriptor-dominated packets that crawl behind the x stream and
    # stall the in-order PE queue at the first decoder matmul.
    # x tails are packed DENSE: patch pl's 16 tail pixels at rows
    # [32*((pl%8)//2) + 16*(pl%2)] of column block pl//8; the K=32 tail
    # matmul covers two patches' rows, with the other patch's 16 rows
    # zeroed in the w operand.
    CW = NP * TM * F // 2       # 1200  w main (f32 cols)
    CWT = NP * F // 2           # 200   w tail (K=32 halves, bf16 -> f32 cols)
    CB = NG                     # 7     bias (real f32)
    CD = NG * OUT // 2          # 35    dec (bf16 pairs)
    CXT = NXB * NB // 4         # 256   x tail (fp8-e3m4)
    CC = CW + CWT + CB + CD + CXT
    c_d = nc.dram_tensor("consts", [128, CC], F32, kind="ExternalInput")
    x_d = nc.dram_tensor("x", [128, NP * SLW], F32, kind="ExternalInput")
    o_d = nc.dram_tensor("out", [OUT, NB], F32, kind="ExternalOutput")

    with tile.TileContext(nc) as tc:
        with (
            tc.tile_pool(name="const", bufs=1) as constp,
            tc.tile_pool(name="xs", bufs=1) as xsp,
            tc.tile_pool(name="yps", bufs=YB, space="PSUM") as ypsp,
            tc.tile_pool(name="ops", bufs=1, space="PSUM") as opsp,
            tc.tile_pool(name="warm", bufs=1, space="PSUM") as warmp,
        ):
            c_sb = constp.tile([128, CC], F32)
            y_sb = constp.tile([128, NG * NB], BF16)
            zero_sb = constp.tile([128, NB], F32)
            out_sb = constp.tile([OUT, NB], F32)
            out_ps = opsp.tile([OUT, NB], F32)

            # consts first on the sync ring: one efficient wide DMA
            nc.sync.dma_start(out=c_sb[:], in_=c_d[:])
            nc.gpsimd.memset(zero_sb[:], 0.0)

            # HAM warm-up: bf16 matmuls on varied (iota) data into scratch
            # PSUM while the PE would otherwise idle waiting for the first x
            # slice. Unthrottles the PE clock gate (1.2 -> 2.4 GHz). fp32 or
            # all-zero matmuls do NOT register as PE activity.
            iota_sb = constp.tile([128, NB], mybir.dt.int32)
            nc.gpsimd.iota(iota_sb[:], [[1, NB]], channel_multiplier=77)
            iota_bf = iota_sb[:].bitcast(BF16)
            warm_ps = warmp.tile([128, 2 * NB], F32)

            def warm(n):
                for _ in range(n):
                    nc.tensor.matmul(
                        warm_ps[0:F, :], iota_bf[:, 0:F], iota_bf[:],
                        start=True, stop=True)

            warm(10)

            # x slices in consumption order; each slice = [fp8 block | bf16
            # block] for its patches, one DMA per slice
            x_tiles = {}
            off = 0
            for g, qlo, qhi in SLICES:
                n = qhi - qlo
                t = xsp.tile([128, n * SLW], F32, name=f"xg{g}_{qlo}")
                nc.sync.dma_start(out=t[:], in_=x_d[:, off:off + n * SLW])
                x_tiles[(g, qlo)] = t
                off += n * SLW
            w_bf = c_sb[:, 0:CW].bitcast(BF16)
            wt_bf = c_sb[:, CW:CW + CWT].bitcast(BF16)
            bias_sb = c_sb[:, CW + CWT:CW + CWT + CB]
            dec_bf = c_sb[:, CW + CWT + CB:CW + CWT + CB + CD].bitcast(BF16)
            xt_bf = c_sb[:, CW + CWT + CB + CD:CC].bitcast(FP8)

            def emit_dec(g, first, last):
                nc.tensor.matmul(
                    out_ps[:],
                    dec_bf[:, g * OUT:(g + 1) * OUT],
                    y_sb[:, g * NB:(g + 1) * NB],
                    start=first,
                    stop=last,
                )

            prev_g = None
            for idx, g in enumerate(ORDER):
                npg = min(4, NP - 4 * g)
                yt = ypsp.tile([128, NB], F32, name="y_ps")
                if idx < YB:
                    # clear stale/NaN PSUM so the gap rows are finite zeros
                    nc.vector.tensor_copy(yt[:], zero_sb[:])
                for g2, qlo, qhi in [s for s in SLICES if s[0] == g]:
                    n = qhi - qlo
                    xg = x_tiles[(g, qlo)]
                    x8 = xg[:, 0:n * NT8 * NB // 4].bitcast(FP8)
                    xm = (xg[:, n * NT8 * NB // 4:n * SLW].bitcast(BF16)
                          if NTM else None)
                    for t in range(TM):
                        for q in range(qlo, qhi):
                            pl = 4 * g + q
                            if t < NT8:
                                rhs = x8[:, ((q - qlo) * NT8 + t) * NB:
                                         ((q - qlo) * NT8 + t + 1) * NB]
                            else:
                                t2 = t - NT8
                                rhs = xm[:, ((q - qlo) * NTM + t2) * NB:
                                         ((q - qlo) * NTM + t2 + 1) * NB]
                            nc.tensor.matmul(
                                yt[32 * q:32 * q + F, :],
                                w_bf[:, (pl * TM + t) * F:
                                     (pl * TM + t + 1) * F],
                                rhs,
                                start=(t == 0),
                                stop=False,
                                tile_position=(0, 32 * q),
                            )
                    for q in range(qlo, qhi):
                        # 16-pixel tail: K=32 matmul over the dense-packed
                        # tail strip (covers this patch + its pair patch,
                        # whose rows are zero in the w operand)
                        pl = 4 * g + q
                        a2, blk = (pl % 8) // 2, pl // 8
                        nc.tensor.matmul(
                            yt[32 * q:32 * q + F, :],
                            wt_bf[32 * a2:32 * a2 + 32, pl * F:(pl + 1) * F],
                            xt_bf[32 * a2:32 * a2 + 32, blk * NB:(blk + 1) * NB],
                            start=False,
                            stop=True,
                            tile_position=(32 * a2, 32 * q),
                        )

                if g != ORDER[-1]:
                    nc.scalar.activation(
                        out=y_sb[:, g * NB:(g + 1) * NB],
                        in_=yt[:],
                        func=mybir.ActivationFunctionType.Relu,
                        bias=bias_sb[:, g:g + 1],
                    )
                    # decoder deferred one group so its y_sb dependency (the
                    # relu above) never stalls the PE behind independent mains
                    if prev_g is not None:
                        emit_dec(prev_g, prev_g == ORDER[0], False)
                    prev_g = g
                    if idx == 0:
                        # bridge the pipeline-fill gap between the first
                        # (tiny) group and the first 4-patch slice so the PE
                        # clock stays warm; later gaps are ~1-2us and real
                        # matmul work keeps it warm (more fillers would
                        # serialize ahead of real MMs)
                        warm(12)
                else:
                    # final group: flush the deferred decoder, then pipeline
                    # relu -> dec -> psum-copy in column halves to shorten
                    # the serial dependency chain at the kernel tail
                    if prev_g is not None:
                        emit_dec(prev_g, prev_g == ORDER[0], False)
                    for h in range(2):
                        cols = slice(g * NB + h * 128, g * NB + h * 128 + 128)
                        nc.scalar.activation(
                            out=y_sb[:, cols],
                            in_=yt[:, h * 128:h * 128 + 128],
                            func=mybir.ActivationFunctionType.Relu,
                            bias=bias_sb[:, g:g + 1],
                        )
                        nc.tensor.matmul(
                            out_ps[:, h * 128:h * 128 + 128],
                            dec_bf[:, g * OUT:(g + 1) * OUT],
                            y_sb[:, cols],
                            start=False,
                            stop=True,
                        )
                        nc.vector.tensor_copy(
                            out_sb[:, h * 128:h * 128 + 128],
                            out_ps[:, h * 128:h * 128 + 128])
                        nc.sync.dma_start(
                            out=o_d[:, h * 128:h * 128 + 128],
                            in_=out_sb[:, h * 128:h * 128 + 128])

    return nc


def make_in_maps(x, weight, bias, dec_w):
    import ml_dtypes
    bf16 = ml_dtypes.bfloat16

    x = np.asarray(x, np.float32).reshape(B, H, W)
    xb = x.astype(bf16)
    # (bh, b, hs, k, ws, l) -> (bh, hs, ws, k, l, b): per-patch pixel-major
    x6 = xb.reshape(2, NB, HS, KS, WS, KS)
    pp = np.ascontiguousarray(x6.transpose(0, 2, 4, 3, 5, 1)).reshape(
        2, P, KS * KS, NB)
    # main chunks: pixel j in [0,768) -> row j%128, col (p, t=j//128, b);
    # chunks t < NT8 quantized to fp8-e3m4, the rest stay bf16
    e3 = ml_dtypes.float8_e3m4
    main = pp[:, :, :TM * 128].reshape(2, P, TM, 128, NB)
    x8h = np.ascontiguousarray(
        main[:, :, :NT8].transpose(0, 3, 1, 2, 4)).astype(e3)
    xmh = np.ascontiguousarray(main[:, :, NT8:].transpose(0, 3, 1, 2, 4))
    tl = pp[:, :, TM * 128:].astype(e3)           # (2, P, 16, NB) fp8 tail

    # weight row j = f*100 + p (pixel-major k*28+l)
    w3 = np.asarray(weight, np.float32).reshape(F, P, KS * KS)
    wm = np.ascontiguousarray(
        w3[:, :, :TM * 128].reshape(F, P, TM, 128).transpose(3, 1, 2, 0)
    ).astype(bf16)                                # (128, P, TM, F)

    b2 = np.asarray(bias, np.float32).reshape(F, P)
    d3 = np.asarray(dec_w, np.float32).reshape(OUT, F, P)

    in_maps = []
    for core in range(NCORES):
        bh, q4 = core // 4, core % 4
        p0 = q4 * NP
        parts = []
        for g, qlo, qhi in SLICES:
            a, b = p0 + 4 * g + qlo, p0 + 4 * g + qhi
            parts.append(
                x8h[bh, :, a:b].reshape(128, -1).view(np.float32))
            if NTM:
                parts.append(
                    xmh[bh, :, a:b].reshape(128, -1).view(np.float32))
        xs = np.concatenate(parts, axis=1)
        ws = np.ascontiguousarray(wm[:, p0:p0 + NP]).reshape(
            128, NP * TM * F)
        xtl = np.zeros((128, NXB * NB), e3)
        wtl = np.zeros((128, NP * F), bf16)
        bst = np.zeros((128, NG), np.float32)
        dst = np.zeros((128, NG * OUT), np.float32)
        for pl in range(NP):
            G, q = pl // 4, pl % 4
            r0 = 32 * ((pl % 8) // 2) + TAIL * (pl % 2)
            xtl[r0:r0 + TAIL, (pl // 8) * NB:(pl // 8 + 1) * NB] = \
                tl[bh, p0 + pl]
            wtl[r0:r0 + TAIL, pl * F:(pl + 1) * F] = \
                w3[:, p0 + pl, TM * 128:].T.astype(bf16)
            bst[32 * q:32 * q + F, G] = b2[:, p0 + pl]
            dst[32 * q:32 * q + F, G * OUT:(G + 1) * OUT] = d3[:, :, p0 + pl].T
        consts = np.concatenate(
            [ws.view(np.float32), wtl.view(np.float32), bst,
             dst.astype(bf16).view(np.float32), xtl.view(np.float32)],
            axis=1)
        in_maps.append({"consts": consts, "x": xs})
    return in_maps


def combine(results, dec_b):
    dec_b = np.asarray(dec_b, np.float32)
    out = np.zeros((B, OUT), np.float32)
    for bh in range(2):
        part = results[4 * bh]["out"].astype(np.float32)
        for q4 in range(1, 4):
            part = part + results[4 * bh + q4]["out"]
        out[bh * NB:(bh + 1) * NB] = part.T + dec_b
    return out


_cache = {}


def _get_nc():
    if "nc" not in _cache:
        nc = build_program()
        nc.finalize()
        _cache["nc"] = nc
    return _cache["nc"]


def _install_ntff_hook():
    """Provide the missing antenv.axon_hooks module so trace=True works
    under axon (replicates trn_boot._ntff_profile_via_ctypes)."""
    import contextlib
    import ctypes
    import types

    if "antenv.axon_hooks" in sys.modules:
        return
    so_path = "/opt/axon/libaxon_pjrt.so"
    holder = {}
    mod = types.ModuleType("antenv.axon_hooks")
    mod.set_axon_ntff_profile_hook = lambda h: holder.__setitem__("h", h)
    mod.get_axon_ntff_profile_hook = lambda: holder.get("h")
    sys.modules["antenv.axon_hooks"] = mod
    try:
        import antenv
        antenv.axon_hooks = mod
    except ImportError:
        pass

    lib = ctypes.CDLL(so_path)
    if not hasattr(lib, "axon_start_nrt_profile"):
        return
    lib.axon_start_nrt_profile.argtypes = [
        ctypes.POINTER(ctypes.c_int64), ctypes.c_size_t]
    lib.axon_start_nrt_profile.restype = ctypes.c_int64
    lib.axon_stop_nrt_profile.argtypes = [ctypes.c_char_p]
    lib.axon_stop_nrt_profile.restype = ctypes.c_int64

    @contextlib.contextmanager
    def _hook(output_dir, device_ids):
        import jax
        jax.devices()
        if device_ids:
            ids = (ctypes.c_int64 * len(device_ids))(*device_ids)
            rc = lib.axon_start_nrt_profile(ids, len(device_ids))
        else:
            rc = lib.axon_start_nrt_profile(None, 0)
        if rc != 0:
            raise RuntimeError(f"axon_start_nrt_profile rc={rc}")
        try:
            yield
        finally:
            n = lib.axon_stop_nrt_profile(str(output_dir).encode())
            print(f"profile: {n} file(s) written to {output_dir}")

    mod.set_axon_ntff_profile_hook(_hook)


def run(x, weight, bias, dec_w, dec_b, trace=False):
    from concourse import bass_utils
    from concourse.bass_utils import run_bass_kernel_spmd

    if trace:
        _install_ntff_hook()
        # artifact upload needs a bucket that doesn't exist here
        bass_utils.upload_artifacts = lambda tmpdir: tmpdir

    nc = _get_nc()
    in_maps = make_in_maps(x, weight, bias, dec_w)
    r = run_bass_kernel_spmd(nc, in_maps, list(range(NCORES)), trace=trace)
    return combine(r.results, dec_b), r


def kernel(x, weight, bias, dec_w, dec_b):
    out, _ = run(x, weight, bias, dec_w, dec_b, trace=False)
    return out
